# revision 9
# baseline (speedup 1.0000x reference)
"""NeRF render kernel for 8 Trainium2 NeuronCores.

Data-parallel over rays: core k handles rays [2048*k, 2048*(k+1)).
Per core: positional encoding + 3-layer MLP (39->256->256->4) over
131072 points in feature-major layout (features on partitions, points
on the free dim), then alpha compositing via triangular-matrix matmul
cumulative sums.

Point permutation inside a core: t_rand rows are loaded as
[128 partitions = ray-group i (rays 16i..16i+15), 1024 = (k, s)] and
PE-transposed per 128-column chunk k0 so that partitions become
q = rp*64 + s (rp = ray parity) and columns J = 128*k0 + i denote the
ray pair (16i + 2*k0, 16i + 2*k0 + 1).  All downstream tiles keep that
column order; the host unscatters at the end.

v2 layout: "super-tiles" of 8 blocks (1024 points) processed as two
4-block chunks A/B mapped to partition bands 0:39 / 64:103, so the
feature transposes run col-tiled pairs and the L0 matmuls run
row-tiled pairs (concurrent in the PE array).  PSUM drains are split
ACT/DVE; SBUF-only elementwise work runs on GPSIMD; compositing is
interleaved per group.
"""

import sys
import numpy as np

sys.path.insert(0, "/opt/trn_rl_repo")

S = 64
L = 6
NCORES = 8
B = 16384
BC = B // NCORES          # rays per core
NP = BC * S               # points per core
NBLK = NP // 128          # 1024 ray-pair blocks
NGRP = 8                  # groups of 128 blocks
HB = 64                   # blocks per half-group
NEAR, FAR = 2.0, 6.0
DELTA = (FAR - NEAR) / S
PI = float(np.pi)
TWO_PI = float(2.0 * np.pi)
INV2PI = float(np.float32(1.0 / (2.0 * np.pi)))
MAGIC = 12582912.0  # 1.5 * 2**23: float32 round-to-int trick
C1 = float(np.float32(2.0 * np.pi))
C2 = float(2.0 * np.pi - np.float64(np.float32(2.0 * np.pi)))

_CACHE = {}
PROFILE = False  # test harness sets True to collect an NTFF trace


def _split_waits(nc, mybir):
    """TRN2 allows one sem wait per instruction (two for EventSemaphore);
    this walrus build rejects over-limit instructions, so move excess waits
    onto chained NOPs on the same engine just before the instruction."""
    ctr = 0
    for fn in nc.m.functions:
        for bb in fn.blocks:
            changed = False
            out = []
            for inst in bb.instructions:
                si = inst.sync_info
                cap = 2 if isinstance(inst, mybir.InstEventSemaphore) else 1
                if si is not None and si.on_wait and len(si.on_wait) > cap:
                    waits = list(si.on_wait)
                    for w in waits[:-cap]:
                        nop = mybir.InstNoOp(
                            name=f"wsplit-{ctr}", ins=[], outs=[]
                        )
                        ctr += 1
                        nop.engine = inst.engine
                        nop.sync_info = mybir.SyncInfo(on_wait=[w], on_update=[])
                        nc.register_instruction(nop)
                        out.append(nop)
                    si.on_wait = waits[-cap:]
                    changed = True
                out.append(inst)
            if changed:
                bb.instructions = out
    return ctr


def _build():
    import concourse.bass as bass
    import concourse.mybir as mybir
    import concourse.tile as tile

    dt = mybir.dt
    AF = mybir.ActivationFunctionType
    OP = mybir.AluOpType
    F32 = dt.float32
    F32R = dt.float32r
    F16 = dt.float16

    nc = bass.Bass()

    # ---- DRAM I/O ----
    tnat_d = nc.dram_tensor("tnat", [128, 1024], F32, kind="ExternalInput")
    aexp_d = nc.dram_tensor("aexp", [3, 128, 1024], F32, kind="ExternalInput")
    bexp_d = nc.dram_tensor("bexp", [3, 128, 1024], F32, kind="ExternalInput")
    w0_d = nc.dram_tensor("w0rep", [128, 256], F16, kind="ExternalInput")
    w1_d = nc.dram_tensor("w1", [256, 256], F16, kind="ExternalInput")
    w2_d = nc.dram_tensor("w2h", [128, 8], F16, kind="ExternalInput")
    b0_d = nc.dram_tensor("b0t", [128, 2], F32, kind="ExternalInput")
    b1_d = nc.dram_tensor("b1t", [128, 2], F32, kind="ExternalInput")
    b2_d = nc.dram_tensor("b2t", [128, 4], F32, kind="ExternalInput")
    zcpp_d = nc.dram_tensor("zcpp", [128, 1], F32, kind="ExternalInput")
    ltri_d = nc.dram_tensor("ltri", [128, 256], F32, kind="ExternalInput")
    sel2_d = nc.dram_tensor("sel2", [128, 2], F32R, kind="ExternalInput")
    ident_d = nc.dram_tensor("ident", [128, 128], F32R, kind="ExternalInput")
    identh_d = nc.dram_tensor("identh", [128, 128], F16, kind="ExternalInput")
    out_d = nc.dram_tensor("out", [NGRP, 2, 384], F32, kind="ExternalOutput")

    with tile.TileContext(nc) as tc:
        with (
            tc.tile_pool(name="consts", bufs=1) as cpool,
            tc.tile_pool(name="tall", bufs=1) as tpool,
            tc.tile_pool(name="o2", bufs=2) as o2pool,
        ):
            # ---- load constants / weights ----
            tnat = cpool.tile([128, 1024], F32, tag="tnat")
            nc.sync.dma_start(tnat[:], tnat_d[:])
            w0rep = cpool.tile([128, 256], F16, tag="w0rep")
            nc.sync.dma_start(w0rep[:], w0_d[:])
            w1s0 = cpool.tile([128, 256], F16, tag="w1s0")
            nc.sync.dma_start(w1s0[:], w1_d[0:128, :])
            w1s1 = cpool.tile([128, 256], F16, tag="w1s1")
            nc.sync.dma_start(w1s1[:], w1_d[128:256, :])
            w2s = cpool.tile([128, 8], F16, tag="w2s")
            nc.sync.dma_start(w2s[:], w2_d[:])
            b0t = cpool.tile([128, 2], F32, tag="b0t")
            nc.sync.dma_start(b0t[:], b0_d[:])
            b1t = cpool.tile([128, 2], F32, tag="b1t")
            nc.sync.dma_start(b1t[:], b1_d[:])
            b2t = cpool.tile([128, 4], F32, tag="b2t")
            nc.sync.dma_start(b2t[:], b2_d[:])
            zcpp = cpool.tile([128, 1], F32, tag="zcpp")
            nc.sync.dma_start(zcpp[:], zcpp_d[:])
            ltri = cpool.tile([128, 256], F32, tag="ltri")
            nc.sync.dma_start(ltri[:], ltri_d[:])
            sel2 = cpool.tile([128, 2], F32R, tag="sel2")
            nc.sync.dma_start(sel2[:], sel2_d[:])
            ident = cpool.tile([128, 128], F32R, tag="ident")
            nc.sync.dma_start(ident[:], ident_d[:])
            identh = cpool.tile([128, 128], F16, tag="identh")
            nc.sync.dma_start(identh[:], identh_d[:])
            zerot = cpool.tile([128, 1], F32, tag="zerot")
            nc.vector.memset(zerot[:], 0.0)

            # ---- phase A: t transpose, z, pts (tall/block layout) ----
            # split heavy ops by column halves between DVE and GPSIMD
            zt = tpool.tile([128, 1024], F32, tag="zt")
            pts3 = tpool.tile([128, 3072], F32, tag="pts3")
            pts3r = tpool.tile([128, 3072], F32, tag="pts3r")
            with (
                tc.tile_pool(name="apool", bufs=1) as apool,
                tc.tile_pool(name="ttp", bufs=2, space="PSUM") as ttp_pool,
            ):
                for k0 in range(8):
                    ttp = ttp_pool.tile([128, 128], F32, tag="ttp")
                    nc.tensor.transpose(
                        ttp[:], tnat[:, 128 * k0 : 128 * (k0 + 1)], ident[:].bitcast(F32)
                    )
                    # z = delta * t + zc[s]
                    nc.vector.tensor_scalar(
                        zt[:, 128 * k0 : 128 * (k0 + 1)],
                        ttp[:],
                        DELTA,
                        zcpp[:, 0:1],
                        op0=OP.mult,
                        op1=OP.add,
                    )
                aexs = []
                bexs = []
                for c in range(3):
                    ae = apool.tile([128, 1024], F32, tag=f"aex{c}")
                    nc.sync.dma_start(ae[:], aexp_d[c])
                    be = apool.tile([128, 1024], F32, tag=f"bex{c}")
                    nc.sync.dma_start(be[:], bexp_d[c])
                    aexs.append(ae)
                    bexs.append(be)
                H = 512
                for c in range(3):
                    pv = pts3.rearrange("p (j c) -> p c j", c=3)[:, c, :]
                    for eng, sl in ((nc.vector, slice(0, H)),
                                    (nc.gpsimd, slice(H, 1024))):
                        eng.tensor_tensor(pv[:, sl], zt[:, sl], bexs[c][:, sl], op=OP.mult)
                        eng.tensor_tensor(pv[:, sl], pv[:, sl], aexs[c][:, sl], op=OP.add)
                # base range reduction: pts3r = pts3 - 2pi*round(pts3/2pi)
                kt = apool.tile([128, 3072], F32, tag="kt")
                H3 = 1536
                for eng, sl in ((nc.vector, slice(0, H3)),
                                (nc.gpsimd, slice(H3, 3072))):
                    eng.tensor_scalar(
                        kt[:, sl], pts3[:, sl], INV2PI, MAGIC, op0=OP.mult, op1=OP.add
                    )
                    eng.tensor_scalar(
                        kt[:, sl], kt[:, sl], MAGIC, -C1, op0=OP.subtract, op1=OP.mult
                    )
                    eng.tensor_tensor(
                        pts3r[:, sl], pts3[:, sl], kt[:, sl], op=OP.add
                    )
                    eng.tensor_scalar(
                        pts3r[:, sl], pts3r[:, sl], PI, -PI, op0=OP.min, op1=OP.max
                    )

            # ---- phase B + C: MLP per super-tile, compositing per group ----
            with (
                tc.tile_pool(name="scr", bufs=2) as scrpool,
                tc.tile_pool(name="kpool", bufs=2) as kpool,
                tc.tile_pool(name="sfp", bufs=2) as sfppool,
                tc.tile_pool(name="fs", bufs=2) as fspool,
                tc.tile_pool(name="h0s", bufs=2) as h0spool,
                tc.tile_pool(name="h1s", bufs=2) as h1spool,
                tc.tile_pool(name="cS", bufs=2) as cspool,
                tc.tile_pool(name="tpP", bufs=1, space="PSUM") as tp_pool,
                tc.tile_pool(name="h0P", bufs=2, space="PSUM") as h0_pool,
                tc.tile_pool(name="h1P", bufs=1, space="PSUM") as h1_pool,
                tc.tile_pool(name="oP", bufs=1, space="PSUM") as o_pool,
            ):
                for g in range(NGRP):
                    og = o_pool.tile([128, 512], F32, tag="og")
                    for half in range(2):
                        hg = 2 * g + half
                        # chain scratch in (freq-feature, block) layout so
                        # every chain op is contiguous (GPSIMD-friendly):
                        # sa2[p, f, j], f: 3l+c sin-arg, 18+3l+c cos-arg.
                        sa = scrpool.tile([128, 36 * HB], F32, tag="sa")
                        sa2 = sa.rearrange("p (f j) -> p f j", j=HB)
                        # pts views iterated (c outer, j inner) to match
                        # the (f, j) destination order
                        p3t = pts3.rearrange(
                            "p (o j c) -> p o c j", o=16, c=3
                        )[:, hg, :, :]
                        p3rt = pts3r.rearrange(
                            "p (o j c) -> p o c j", o=16, c=3
                        )[:, hg, :, :]
                        nc.vector.tensor_copy(sa2[:, 0:3, :], p3rt)
                        # sin l: r_l = 2 r_{l-1} - 2pi*round(2 r_{l-1}/2pi)
                        # (gpsimd has no scalar_tensor_tensor: compute
                        #  half = r - pi*k, then double)
                        for l in range(1, L):
                            prev = sa2[:, 3 * (l - 1) : 3 * l, :]
                            cur = sa2[:, 3 * l : 3 * l + 3, :]
                            kb = kpool.tile([128, HB * 3], F32, tag="kb")
                            kbv = kb.rearrange("p (c j) -> p c j", j=HB)
                            nc.gpsimd.tensor_scalar(
                                kbv, prev, 2.0 * INV2PI, MAGIC,
                                op0=OP.mult, op1=OP.add,
                            )
                            nc.gpsimd.tensor_scalar(
                                kbv, kbv, MAGIC, -PI,
                                op0=OP.subtract, op1=OP.mult,
                            )
                            nc.gpsimd.tensor_tensor(cur, prev, kbv, op=OP.add)
                            nc.gpsimd.tensor_scalar(
                                cur, cur, 2.0, None, op0=OP.mult
                            )
                        # cos l: c_l = (r_l + pi/2) - 2pi*[r_l > pi/2]
                        for l in range(L):
                            rl = sa2[:, 3 * l : 3 * l + 3, :]
                            cl = sa2[:, 18 + 3 * l : 21 + 3 * l, :]
                            kb = kpool.tile([128, HB * 3], F32, tag="kb")
                            kbv = kb.rearrange("p (c j) -> p c j", j=HB)
                            nc.gpsimd.tensor_scalar(
                                kbv, rl, PI / 2, -C1, op0=OP.is_gt, op1=OP.mult
                            )
                            nc.gpsimd.tensor_tensor(cl, rl, kbv, op=OP.add)
                            nc.gpsimd.tensor_scalar(
                                cl, cl, PI / 2, None, op0=OP.add
                            )
                        # trig + raw pts into the fp16 staging tile; the
                        # (j, f)-ordered output AP matches the (f, j) input
                        # iteration order (f outer, j inner)
                        sf = sfppool.tile([128, HB * 39], F16, tag="sf")
                        sff = sf.rearrange("p (j f) -> p f j", f=39)
                        nc.scalar.activation(
                            sff[:, 0:36, :], sa2[:, 0:36, :], AF.Sin
                        )
                        nc.gpsimd.tensor_copy(sff[:, 36:39, :], p3t)
                        # super-tiles: 8 blocks = chunk A (4) + chunk B (4)
                        for st in range(HB // 8):
                            # PE-transpose col-tiled pairs: A -> band 0:39,
                            # B -> band 64:103 of one fp16 PSUM tile
                            tp = tp_pool.tile([104, 512], F16, tag="tp")
                            for k in range(4):
                                jA = 8 * st + k
                                jB = 8 * st + 4 + k
                                nc.tensor.transpose(
                                    tp[0:39, 128 * k : 128 * (k + 1)],
                                    sf[:, 39 * jA : 39 * (jA + 1)],
                                    identh[:],
                                )
                                nc.tensor.transpose(
                                    tp[64:103, 128 * k : 128 * (k + 1)],
                                    sf[:, 39 * jB : 39 * (jB + 1)],
                                    identh[:],
                                )
                            # single fused copy (rows 39:64 are dead junk,
                            # never read downstream); alternate engine
                            fs = fspool.tile([104, 512], F16, tag="fs")
                            if st % 2 == 0:
                                nc.scalar.activation(
                                    fs[0:103, :], tp[0:103, :], AF.Copy
                                )
                            else:
                                nc.vector.tensor_copy(
                                    fs[0:103, :], tp[0:103, :]
                                )
                            # L0: row-tiled pairs (bands 0 / 64); PSUM tile
                            # per hidden-half holds [chunk A | chunk B] so
                            # the drain is one FD-1024 op with uniform bias
                            h0ss = []
                            for h in range(2):
                                h0p = h0_pool.tile([128, 1024], F32, tag="h0p")
                                for x in range(2):
                                    lo = 64 * x
                                    nc.tensor.matmul(
                                        h0p[:, 512 * x : 512 * (x + 1)],
                                        w0rep[lo : lo + 39, 128 * h : 128 * (h + 1)],
                                        fs[lo : lo + 39, :],
                                    )
                                h0s = h0spool.tile([128, 1024], F16, tag="h0s")
                                if h == 0:
                                    nc.scalar.activation(
                                        h0s[:], h0p[:], AF.Relu,
                                        bias=b0t[:, 0:1],
                                    )
                                else:
                                    nc.vector.tensor_scalar(
                                        h0s[:], h0p[:],
                                        b0t[:, 1:2], 0.0, op0=OP.add, op1=OP.max,
                                    )
                                h0ss.append(h0s)
                            # L1: PSUM tile per out-half g = [A | B]
                            h1ss = []
                            for gh in range(2):
                                h1p = h1_pool.tile([128, 1024], F32, tag="h1p")
                                for x in range(2):
                                    nc.tensor.matmul(
                                        h1p[:, 512 * x : 512 * (x + 1)],
                                        w1s0[:, 128 * gh : 128 * (gh + 1)],
                                        h0ss[0][:, 512 * x : 512 * (x + 1)],
                                        start=True,
                                        stop=False,
                                    )
                                    nc.tensor.matmul(
                                        h1p[:, 512 * x : 512 * (x + 1)],
                                        w1s1[:, 128 * gh : 128 * (gh + 1)],
                                        h0ss[1][:, 512 * x : 512 * (x + 1)],
                                        start=False,
                                        stop=True,
                                    )
                                h1s = h1spool.tile([128, 1024], F16, tag="h1s")
                                if gh == 0:
                                    nc.scalar.activation(
                                        h1s[:], h1p[:], AF.Relu,
                                        bias=b1t[:, 0:1],
                                    )
                                else:
                                    nc.vector.tensor_scalar(
                                        h1s[:], h1p[:],
                                        b1t[:, 1:2], 0.0,
                                        op0=OP.add, op1=OP.max,
                                    )
                                h1ss.append(h1s)
                            # L2: activations stationary, W2 moving
                            for x in range(2):
                                for jp in range(4):
                                    jj = HB * half + 8 * st + 4 * x + jp
                                    nc.tensor.matmul(
                                        og[:, 4 * jj : 4 * (jj + 1)],
                                        h1ss[0][:, 512 * x + 128 * jp : 512 * x + 128 * (jp + 1)],
                                        w2s[:, 0:4],
                                        start=True,
                                        stop=False,
                                    )
                                    nc.tensor.matmul(
                                        og[:, 4 * jj : 4 * (jj + 1)],
                                        h1ss[1][:, 512 * x + 128 * jp : 512 * x + 128 * (jp + 1)],
                                        w2s[:, 4:8],
                                        start=False,
                                        stop=True,
                                    )
                    # ---- drain O psum -> O2 sbuf (per-channel bias ops,
                    # relu on sigma; split ACT/DVE) ----
                    o2 = o2pool.tile([128, 512], F32, tag="o2")
                    ogv = og.rearrange("p (j c) -> p j c", c=4)
                    o2v = o2.rearrange("p (j c) -> p j c", c=4)
                    nc.scalar.activation(
                        o2v[:, :, 0], ogv[:, :, 0], AF.Identity, bias=b2t[:, 0:1]
                    )
                    nc.vector.tensor_scalar(
                        o2v[:, :, 1], ogv[:, :, 1], b2t[:, 1:2], None, op0=OP.add
                    )
                    nc.scalar.activation(
                        o2v[:, :, 2], ogv[:, :, 2], AF.Identity, bias=b2t[:, 2:3]
                    )
                    nc.vector.tensor_scalar(
                        o2v[:, :, 3], ogv[:, :, 3], b2t[:, 3:4], 0.0,
                        op0=OP.add, op1=OP.max,
                    )
                    # ---- compositing for this group (interleaved) ----
                    e = cspool.tile([128, 384], F32, tag="e")
                    nc.scalar.activation(
                        e.rearrange("p (j c) -> p j c", c=3),
                        o2v[:, :, 0:3],
                        AF.Sigmoid,
                    )
                    # scans: exclusive & inclusive cumsum of sigma over s
                    # (ct borrows the h1 PSUM ring)
                    ct = h1_pool.tile([128, 1024], F32, tag="h1p")
                    sig = o2v[:, :, 3]
                    nc.tensor.matmul(ct[:, 0:128], ltri[:, 0:128], sig)
                    nc.tensor.matmul(ct[:, 128:256], ltri[:, 128:256], sig)
                    texin = cspool.tile([128, 256], F32, tag="texin")
                    nc.scalar.activation(texin[:], ct[:, 0:256], AF.Exp, scale=-DELTA)
                    wt = cspool.tile([128, 128], F32, tag="wt")
                    nc.gpsimd.tensor_tensor(
                        wt[:], texin[:, 0:128], texin[:, 128:256], op=OP.subtract
                    )
                    wr = cspool.tile([128, 384], F32R, tag="wr")
                    nc.gpsimd.tensor_tensor(
                        wr.rearrange("p (j c) -> p j c", c=3),
                        e.rearrange("p (j c) -> p j c", c=3),
                        wt.unsqueeze(2).broadcast_to([128, 128, 3]),
                        op=OP.mult,
                    )
                    # final per-ray-parity sum into spare cols of ct's bank
                    rp_ = ct[0:2, 128:512]
                    nc.tensor.matmul(rp_, sel2[:], wr[:])
                    outs = cspool.tile([2, 384], F32, tag="outs")
                    nc.vector.tensor_copy(outs[:], rp_)
                    nc.sync.dma_start(out_d[g], outs[:])

    _split_waits(nc, mybir)
    return nc


def _host_prep(origins, directions, t_rand, W0, b0, W1, b1, W2, b2):
    """Build per-core input maps (all numpy, cheap)."""
    f32 = np.float32
    # F-row order: rows 3l+c = sin freq l coord c; 18+3l+c = cos; 36..38 pts
    perm = np.zeros(39, np.int64)
    perm[36:39] = (0, 1, 2)
    for l in range(L):
        for c in range(3):
            perm[3 * l + c] = 3 + 6 * l + c
            perm[18 + 3 * l + c] = 3 + 6 * l + 3 + c
    w0p = np.ascontiguousarray(W0[perm]).astype(np.float16)
    w0rep = np.zeros((128, 256), np.float16)
    w0rep[0:39] = w0p
    w0rep[64:103] = w0p

    w2h = np.empty((128, 8), np.float16)
    w2h[:, 0:4] = W2[0:128].astype(np.float16)
    w2h[:, 4:8] = W2[128:256].astype(np.float16)
    b0t = np.ascontiguousarray(b0.reshape(2, 128).T).astype(f32)
    b1t = np.ascontiguousarray(b1.reshape(2, 128).T).astype(f32)
    b2t = np.broadcast_to(b2.astype(f32), (128, 4)).copy()

    q = np.arange(128)
    rp = q // 64
    s = q % 64
    zcpp = (NEAR + DELTA * s).astype(f32).reshape(128, 1).copy()

    # ltri: cols 0..127 exclusive, 128..255 inclusive
    # ltri[k=(rp',j), m=(rp,s)] = (rp'==rp) & (j < s)  /  (j <= s)
    kk = q
    krp = kk // 64
    kj = kk % 64
    same = (krp[:, None] == rp[None, :])
    ltri = np.zeros((128, 256), f32)
    ltri[:, 0:128] = (same & (kj[:, None] < s[None, :])).astype(f32)
    ltri[:, 128:256] = (same & (kj[:, None] <= s[None, :])).astype(f32)
    sel2 = (krp[:, None] == np.arange(2)[None, :]).astype(f32)
    ident = np.eye(128, dtype=f32)
    identh = np.eye(128, dtype=np.float16)

    # ray_of[J, rp] = 16*(J%128) + 2*(J//128) + rp
    J = np.arange(NBLK)
    ray_of = (16 * (J % 128))[:, None] + (2 * (J // 128))[:, None] + np.arange(2)[None, :]

    in_maps = []
    for core in range(NCORES):
        o = origins[core * BC : (core + 1) * BC].astype(f32)
        d = directions[core * BC : (core + 1) * BC].astype(f32)
        t = t_rand[core * BC : (core + 1) * BC].astype(f32)
        tnat = np.ascontiguousarray(t.reshape(128, 1024))
        # aexp[c, q, J] = o[ray_of[J, rp(q)], c]
        rays_qJ = ray_of[:, :].T[rp]  # [128, NBLK] -> rays_qJ[q, J] = ray_of[J, rp[q]]
        aexp = np.ascontiguousarray(o[rays_qJ].transpose(2, 0, 1))
        bexp = np.ascontiguousarray(d[rays_qJ].transpose(2, 0, 1))
        in_maps.append(
            {
                "tnat": tnat,
                "aexp": aexp,
                "bexp": bexp,
                "w0rep": w0rep,
                "w1": W1.astype(np.float16),
                "w2h": w2h,
                "b0t": b0t,
                "b1t": b1t,
                "b2t": b2t,
                "zcpp": zcpp,
                "ltri": ltri,
                "sel2": sel2,
                "ident": ident,
                "identh": identh,
            }
        )
    return in_maps, ray_of


def kernel(origins, directions, t_rand, W0, b0, W1, b1, W2, b2, near, far,
           **kw):
    assert int(near) == 2 and int(far) == 6
    from concourse.bass_utils import run_bass_kernel_spmd

    if "nc" not in _CACHE:
        _CACHE["nc"] = _build()
    nc = _CACHE["nc"]

    in_maps, ray_of = _host_prep(
        np.asarray(origins), np.asarray(directions), np.asarray(t_rand),
        np.asarray(W0), np.asarray(b0), np.asarray(W1), np.asarray(b1),
        np.asarray(W2), np.asarray(b2),
    )
    res = run_bass_kernel_spmd(
        nc, in_maps, core_ids=list(range(NCORES)), trace=PROFILE
    )
    _CACHE["last_results"] = res
    out = np.empty((B, 3), np.float32)
    for core in range(NCORES):
        oc = res.results[core]["out"].reshape(NGRP, 2, 128, 3)
        # group g holds blocks J = 128*g + i ; ray = 16*i + 2*g + rp
        for g in range(NGRP):
            for rpp in range(2):
                rays = core * BC + 16 * np.arange(128) + 2 * g + rpp
                out[rays] = oc[g, rpp]
    return out


# revision 13
# speedup vs baseline: 1.5132x; 1.5132x over previous
"""NeRF render kernel for 8 Trainium2 NeuronCores.

Data-parallel over rays: core k handles rays [2048*k, 2048*(k+1)).
Per core: positional encoding + 3-layer MLP (39->256->256->4) over
131072 points in feature-major layout (features on partitions, points
on the free dim), then alpha compositing via triangular-matrix matmul
cumulative sums.

Point permutation inside a core: t_rand rows are loaded as
[128 partitions = ray-group i (rays 16i..16i+15), 1024 = (k, s)] and
PE-transposed per 128-column chunk k0 so that partitions become
q = rp*64 + s (rp = ray parity) and columns J = 128*k0 + i denote the
ray pair (16i + 2*k0, 16i + 2*k0 + 1).  All downstream tiles keep that
column order; the host unscatters at the end.

v2 layout: "super-tiles" of 8 blocks (1024 points) processed as two
4-block chunks A/B mapped to partition bands 0:39 / 64:103, so the
feature transposes run col-tiled pairs and the L0 matmuls run
row-tiled pairs (concurrent in the PE array).  PSUM drains are split
ACT/DVE; SBUF-only elementwise work runs on GPSIMD; compositing is
interleaved per group.
"""

import sys
import numpy as np

sys.path.insert(0, "/opt/trn_rl_repo")

S = 64
L = 6
NCORES = 8
B = 16384
BC = B // NCORES          # rays per core
NP = BC * S               # points per core
NBLK = NP // 128          # 1024 ray-pair blocks
NGRP = 8                  # groups of 128 blocks
HB = 64                   # blocks per half-group
NEAR, FAR = 2.0, 6.0
DELTA = (FAR - NEAR) / S
PI = float(np.pi)
TWO_PI = float(2.0 * np.pi)
INV2PI = float(np.float32(1.0 / (2.0 * np.pi)))
MAGIC = 12582912.0  # 1.5 * 2**23: float32 round-to-int trick
C1 = float(np.float32(2.0 * np.pi))
C2 = float(2.0 * np.pi - np.float64(np.float32(2.0 * np.pi)))

_CACHE = {}
PROFILE = False  # test harness sets True to collect an NTFF trace


def _split_waits(nc, mybir):
    """TRN2 allows one sem wait per instruction (two for EventSemaphore);
    this walrus build rejects over-limit instructions, so move excess waits
    onto chained NOPs on the same engine just before the instruction."""
    ctr = 0
    for fn in nc.m.functions:
        for bb in fn.blocks:
            changed = False
            out = []
            for inst in bb.instructions:
                si = inst.sync_info
                cap = 2 if isinstance(inst, mybir.InstEventSemaphore) else 1
                if si is not None and si.on_wait and len(si.on_wait) > cap:
                    waits = list(si.on_wait)
                    for w in waits[:-cap]:
                        nop = mybir.InstNoOp(
                            name=f"wsplit-{ctr}", ins=[], outs=[]
                        )
                        ctr += 1
                        nop.engine = inst.engine
                        nop.sync_info = mybir.SyncInfo(on_wait=[w], on_update=[])
                        nc.register_instruction(nop)
                        out.append(nop)
                    si.on_wait = waits[-cap:]
                    changed = True
                out.append(inst)
            if changed:
                bb.instructions = out
    return ctr


def _build():
    import concourse.bass as bass
    import concourse.mybir as mybir
    import concourse.tile as tile

    dt = mybir.dt
    AF = mybir.ActivationFunctionType
    OP = mybir.AluOpType
    F32 = dt.float32
    F32R = dt.float32r
    F16 = dt.float16

    nc = bass.Bass()

    # ---- DRAM I/O ----
    tnat_d = nc.dram_tensor("tnat", [128, 1024], F32, kind="ExternalInput")
    aexp_d = nc.dram_tensor("aexp", [3, 128, 1024], F32, kind="ExternalInput")
    bexp_d = nc.dram_tensor("bexp", [3, 128, 1024], F32, kind="ExternalInput")
    w0_d = nc.dram_tensor("w0rep", [128, 256], F16, kind="ExternalInput")
    w1_d = nc.dram_tensor("w1", [256, 256], F16, kind="ExternalInput")
    w2_d = nc.dram_tensor("w2h", [128, 8], F16, kind="ExternalInput")
    b0_d = nc.dram_tensor("b0t", [128, 2], F32, kind="ExternalInput")
    b1_d = nc.dram_tensor("b1t", [128, 2], F32, kind="ExternalInput")
    b2_d = nc.dram_tensor("b2t", [128, 4], F32, kind="ExternalInput")
    zcpp_d = nc.dram_tensor("zcpp", [128, 1], F32, kind="ExternalInput")
    ltri_d = nc.dram_tensor("ltri", [128, 256], F32, kind="ExternalInput")
    sel2_d = nc.dram_tensor("sel2", [128, 2], F32R, kind="ExternalInput")
    ident_d = nc.dram_tensor("ident", [128, 128], F32R, kind="ExternalInput")
    identh_d = nc.dram_tensor("identh", [128, 128], F16, kind="ExternalInput")
    out_d = nc.dram_tensor("out", [NGRP, 2, 384], F32, kind="ExternalOutput")

    with tile.TileContext(nc) as tc:
        with (
            tc.tile_pool(name="consts", bufs=1) as cpool,
            tc.tile_pool(name="tall", bufs=1) as tpool,
            tc.tile_pool(name="o2", bufs=2) as o2pool,
        ):
            # ---- load constants / weights ----
            tnat = cpool.tile([128, 1024], F32, tag="tnat")
            nc.sync.dma_start(tnat[:], tnat_d[:])
            w0rep = cpool.tile([128, 256], F16, tag="w0rep")
            nc.sync.dma_start(w0rep[:], w0_d[:])
            w1s0 = cpool.tile([128, 256], F16, tag="w1s0")
            nc.sync.dma_start(w1s0[:], w1_d[0:128, :])
            w1s1 = cpool.tile([128, 256], F16, tag="w1s1")
            nc.sync.dma_start(w1s1[:], w1_d[128:256, :])
            w2s = cpool.tile([128, 8], F16, tag="w2s")
            nc.sync.dma_start(w2s[:], w2_d[:])
            b0t = cpool.tile([128, 2], F32, tag="b0t")
            nc.sync.dma_start(b0t[:], b0_d[:])
            b1t = cpool.tile([128, 2], F32, tag="b1t")
            nc.sync.dma_start(b1t[:], b1_d[:])
            b2t = cpool.tile([128, 4], F32, tag="b2t")
            nc.sync.dma_start(b2t[:], b2_d[:])
            zcpp = cpool.tile([128, 1], F32, tag="zcpp")
            nc.sync.dma_start(zcpp[:], zcpp_d[:])
            ltri = cpool.tile([128, 256], F32, tag="ltri")
            nc.sync.dma_start(ltri[:], ltri_d[:])
            sel2 = cpool.tile([128, 2], F32R, tag="sel2")
            nc.sync.dma_start(sel2[:], sel2_d[:])
            ident = cpool.tile([128, 128], F32R, tag="ident")
            nc.sync.dma_start(ident[:], ident_d[:])
            identh = cpool.tile([128, 128], F16, tag="identh")
            nc.sync.dma_start(identh[:], identh_d[:])
            zerot = cpool.tile([128, 1], F32, tag="zerot")
            nc.vector.memset(zerot[:], 0.0)

            # ---- phase A: t transpose, z, pts (tall/block layout) ----
            # split heavy ops by column halves between DVE and GPSIMD
            zt = tpool.tile([128, 1024], F32, tag="zt")
            pts3 = tpool.tile([128, 3072], F32, tag="pts3")
            pts3r = tpool.tile([128, 3072], F32, tag="pts3r")
            with (
                tc.tile_pool(name="apool", bufs=1) as apool,
                tc.tile_pool(name="ttp", bufs=2, space="PSUM") as ttp_pool,
            ):
                for k0 in range(8):
                    ttp = ttp_pool.tile([128, 128], F32, tag="ttp")
                    nc.tensor.transpose(
                        ttp[:], tnat[:, 128 * k0 : 128 * (k0 + 1)], ident[:].bitcast(F32)
                    )
                    # z = delta * t + zc[s]
                    nc.vector.tensor_scalar(
                        zt[:, 128 * k0 : 128 * (k0 + 1)],
                        ttp[:],
                        DELTA,
                        zcpp[:, 0:1],
                        op0=OP.mult,
                        op1=OP.add,
                    )
                aexs = []
                bexs = []
                for c in range(3):
                    ae = apool.tile([128, 1024], F32, tag=f"aex{c}")
                    nc.sync.dma_start(ae[:], aexp_d[c])
                    be = apool.tile([128, 1024], F32, tag=f"bex{c}")
                    nc.sync.dma_start(be[:], bexp_d[c])
                    aexs.append(ae)
                    bexs.append(be)
                H = 512
                for c in range(3):
                    pv = pts3.rearrange("p (j c) -> p c j", c=3)[:, c, :]
                    for eng, sl in ((nc.vector, slice(0, H)),
                                    (nc.gpsimd, slice(H, 1024))):
                        eng.tensor_tensor(pv[:, sl], zt[:, sl], bexs[c][:, sl], op=OP.mult)
                        eng.tensor_tensor(pv[:, sl], pv[:, sl], aexs[c][:, sl], op=OP.add)
                # base range reduction: pts3r = pts3 - 2pi*round(pts3/2pi)
                kt = apool.tile([128, 3072], F32, tag="kt")
                H3 = 1536
                for eng, sl in ((nc.vector, slice(0, H3)),
                                (nc.gpsimd, slice(H3, 3072))):
                    eng.tensor_scalar(
                        kt[:, sl], pts3[:, sl], INV2PI, MAGIC, op0=OP.mult, op1=OP.add
                    )
                    eng.tensor_scalar(
                        kt[:, sl], kt[:, sl], MAGIC, -C1, op0=OP.subtract, op1=OP.mult
                    )
                    eng.tensor_tensor(
                        pts3r[:, sl], pts3[:, sl], kt[:, sl], op=OP.add
                    )
                    eng.tensor_scalar(
                        pts3r[:, sl], pts3r[:, sl], PI, -PI, op0=OP.min, op1=OP.max
                    )

            # ---- phase B + C: MLP per super-tile, compositing per group ----
            with (
                tc.tile_pool(name="scr", bufs=2) as scrpool,
                tc.tile_pool(name="kpool", bufs=2) as kpool,
                tc.tile_pool(name="sfp", bufs=2) as sfppool,
                tc.tile_pool(name="fs", bufs=2) as fspool,
                tc.tile_pool(name="h0s", bufs=2) as h0spool,
                tc.tile_pool(name="h1s", bufs=2) as h1spool,
                tc.tile_pool(name="cS", bufs=2) as cspool,
                tc.tile_pool(name="tpP", bufs=1, space="PSUM") as tp_pool,
                tc.tile_pool(name="h0P", bufs=2, space="PSUM") as h0_pool,
                tc.tile_pool(name="h1P", bufs=1, space="PSUM") as h1_pool,
                tc.tile_pool(name="oP", bufs=1, space="PSUM") as o_pool,
            ):
                for g in range(NGRP):
                    og = o_pool.tile([128, 512], F32, tag="og")
                    for half in range(2):
                        hg = 2 * g + half
                        # sin-arg chain in (freq-feature, block) layout:
                        # sa2[p, f, j] rows 0:3 = x/2, 3:6 = x, 3(l+1)+c =
                        # wrapped 2^l x.  Cos features come later from the
                        # identity cos(2a) = 1 - 2 sin^2(a), so only sin
                        # args are chained.
                        W = 3 * HB  # cols per feature-triple row-group
                        sa = scrpool.tile([128, 7 * W], F32, tag="sa")
                        # pts views iterated (c outer, j inner) to match
                        # the (f, j) destination order
                        p3t = pts3.rearrange(
                            "p (o j c) -> p o c j", o=16, c=3
                        )[:, hg, :, :]
                        p3rt = pts3r.rearrange(
                            "p (o j c) -> p o c j", o=16, c=3
                        )[:, hg, :, :]
                        nc.vector.tensor_scalar(
                            sa.rearrange("p (f c j) -> p f c j", f=7, c=3)[:, 0],
                            p3rt, 0.5, None, op0=OP.mult,
                        )
                        nc.vector.tensor_copy(
                            sa.rearrange("p (f c j) -> p f c j", f=7, c=3)[:, 1],
                            p3rt,
                        )
                        # r_l = 2 r_{l-1} - 2pi*round(r_{l-1}/pi):
                        # k-scale on DVE (contiguous, 2x mode), the
                        # tensor_tensor on gpsimd, double on DVE
                        for l in range(1, L):
                            prev = sa[:, l * W : (l + 1) * W]
                            cur = sa[:, (l + 1) * W : (l + 2) * W]
                            kb = kpool.tile([128, W], F32, tag="kb")
                            nc.vector.tensor_scalar(
                                kb[:], prev, 2.0 * INV2PI, MAGIC,
                                op0=OP.mult, op1=OP.add,
                            )
                            nc.vector.tensor_scalar(
                                kb[:], kb[:], MAGIC, -PI,
                                op0=OP.subtract, op1=OP.mult,
                            )
                            nc.gpsimd.tensor_tensor(cur, prev, kb[:], op=OP.add)
                            nc.vector.tensor_scalar(
                                cur, cur, 2.0, None, op0=OP.mult
                            )
                        # staging tile (f, j): rows 0:18 sin, 18:36 cos,
                        # 36:39 raw pts, 39:42 sin(x/2) scratch
                        sf = sfppool.tile([128, 42 * HB], F16, tag="sf")
                        nc.scalar.activation(
                            sf[:, 0 : 6 * W], sa[:, W : 7 * W], AF.Sin
                        )
                        nc.scalar.activation(
                            sf[:, 13 * W : 14 * W], sa[:, 0:W], AF.Sin
                        )
                        # cos_l = 1 - 2 sin(2^{l-1} x)^2   (fp16, DVE)
                        for l in range(L):
                            src = (sf[:, 13 * W : 14 * W] if l == 0
                                   else sf[:, (l - 1) * W : l * W])
                            kb16 = kpool.tile([128, W], F16, tag="kb16")
                            nc.vector.tensor_tensor(kb16[:], src, src, op=OP.mult)
                            nc.vector.tensor_scalar(
                                sf[:, (6 + l) * W : (7 + l) * W],
                                kb16[:], -2.0, 1.0, op0=OP.mult, op1=OP.add,
                            )
                        nc.vector.tensor_copy(
                            sf.rearrange("p (f c j) -> p f c j", f=14, c=3)[:, 12],
                            p3t,
                        )
                        # super-tiles: 8 blocks = chunk A (4) + chunk B (4)
                        sfT = sf.rearrange("p (f j) -> p f j", j=HB)
                        for st in range(HB // 8):
                            # PE-transpose col-tiled pairs: A -> band 0:39,
                            # B -> band 64:103 of one fp16 PSUM tile
                            # (strided lhsT column-gather from the (f, j)
                            # staging layout; LDW cost +30% only)
                            tp = tp_pool.tile([104, 512], F16, tag="tp")
                            for k in range(4):
                                jA = 8 * st + k
                                jB = 8 * st + 4 + k
                                nc.tensor.transpose(
                                    tp[0:39, 128 * k : 128 * (k + 1)],
                                    sfT[:, 0:39, jA],
                                    identh[:],
                                )
                                nc.tensor.transpose(
                                    tp[64:103, 128 * k : 128 * (k + 1)],
                                    sfT[:, 0:39, jB],
                                    identh[:],
                                )
                            # single fused copy (rows 39:64 are dead junk,
                            # never read downstream); alternate engine
                            fs = fspool.tile([104, 512], F16, tag="fs")
                            if st % 2 == 0:
                                nc.scalar.activation(
                                    fs[0:103, :], tp[0:103, :], AF.Copy
                                )
                            else:
                                nc.vector.tensor_copy(
                                    fs[0:103, :], tp[0:103, :]
                                )
                            # L0: row-tiled pairs (bands 0 / 64); PSUM tile
                            # per hidden-half holds [chunk A | chunk B] so
                            # the drain is one FD-1024 op with uniform bias
                            h0ss = []
                            for h in range(2):
                                h0p = h0_pool.tile([128, 1024], F32, tag="h0p")
                                for x in range(2):
                                    lo = 64 * x
                                    nc.tensor.matmul(
                                        h0p[:, 512 * x : 512 * (x + 1)],
                                        w0rep[lo : lo + 39, 128 * h : 128 * (h + 1)],
                                        fs[lo : lo + 39, :],
                                    )
                                h0s = h0spool.tile([128, 1024], F16, tag="h0s")
                                if h == 0:
                                    nc.scalar.activation(
                                        h0s[:], h0p[:], AF.Relu,
                                        bias=b0t[:, 0:1],
                                    )
                                else:
                                    nc.vector.tensor_scalar(
                                        h0s[:], h0p[:],
                                        b0t[:, 1:2], 0.0, op0=OP.add, op1=OP.max,
                                    )
                                h0ss.append(h0s)
                            # L1: PSUM tile per out-half g = [A | B]
                            h1ss = []
                            for gh in range(2):
                                h1p = h1_pool.tile([128, 1024], F32, tag="h1p")
                                for x in range(2):
                                    nc.tensor.matmul(
                                        h1p[:, 512 * x : 512 * (x + 1)],
                                        w1s0[:, 128 * gh : 128 * (gh + 1)],
                                        h0ss[0][:, 512 * x : 512 * (x + 1)],
                                        start=True,
                                        stop=False,
                                    )
                                    nc.tensor.matmul(
                                        h1p[:, 512 * x : 512 * (x + 1)],
                                        w1s1[:, 128 * gh : 128 * (gh + 1)],
                                        h0ss[1][:, 512 * x : 512 * (x + 1)],
                                        start=False,
                                        stop=True,
                                    )
                                h1s = h1spool.tile([128, 1024], F16, tag="h1s")
                                if gh == 0:
                                    nc.scalar.activation(
                                        h1s[:], h1p[:], AF.Relu,
                                        bias=b1t[:, 0:1],
                                    )
                                else:
                                    nc.vector.tensor_scalar(
                                        h1s[:], h1p[:],
                                        b1t[:, 1:2], 0.0,
                                        op0=OP.add, op1=OP.max,
                                    )
                                h1ss.append(h1s)
                            # L2: activations stationary, W2 moving
                            for x in range(2):
                                for jp in range(4):
                                    jj = HB * half + 8 * st + 4 * x + jp
                                    nc.tensor.matmul(
                                        og[:, 4 * jj : 4 * (jj + 1)],
                                        h1ss[0][:, 512 * x + 128 * jp : 512 * x + 128 * (jp + 1)],
                                        w2s[:, 0:4],
                                        start=True,
                                        stop=False,
                                    )
                                    nc.tensor.matmul(
                                        og[:, 4 * jj : 4 * (jj + 1)],
                                        h1ss[1][:, 512 * x + 128 * jp : 512 * x + 128 * (jp + 1)],
                                        w2s[:, 4:8],
                                        start=False,
                                        stop=True,
                                    )
                    # ---- drain O psum -> O2 sbuf (per-channel bias ops,
                    # relu on sigma; split ACT/DVE) ----
                    o2 = o2pool.tile([128, 512], F32, tag="o2")
                    ogv = og.rearrange("p (j c) -> p j c", c=4)
                    o2v = o2.rearrange("p (j c) -> p j c", c=4)
                    nc.scalar.activation(
                        o2v[:, :, 0], ogv[:, :, 0], AF.Identity, bias=b2t[:, 0:1]
                    )
                    nc.vector.tensor_scalar(
                        o2v[:, :, 1], ogv[:, :, 1], b2t[:, 1:2], None, op0=OP.add
                    )
                    nc.scalar.activation(
                        o2v[:, :, 2], ogv[:, :, 2], AF.Identity, bias=b2t[:, 2:3]
                    )
                    nc.vector.tensor_scalar(
                        o2v[:, :, 3], ogv[:, :, 3], b2t[:, 3:4], 0.0,
                        op0=OP.add, op1=OP.max,
                    )
                    # ---- compositing for this group (interleaved) ----
                    e = cspool.tile([128, 384], F32, tag="e")
                    nc.scalar.activation(
                        e.rearrange("p (j c) -> p j c", c=3),
                        o2v[:, :, 0:3],
                        AF.Sigmoid,
                    )
                    # scans: exclusive & inclusive cumsum of sigma over s
                    # (ct borrows the h1 PSUM ring)
                    ct = h1_pool.tile([128, 1024], F32, tag="h1p")
                    sig = o2v[:, :, 3]
                    nc.tensor.matmul(ct[:, 0:128], ltri[:, 0:128], sig)
                    nc.tensor.matmul(ct[:, 128:256], ltri[:, 128:256], sig)
                    texin = cspool.tile([128, 256], F32, tag="texin")
                    nc.scalar.activation(texin[:], ct[:, 0:256], AF.Exp, scale=-DELTA)
                    wt = cspool.tile([128, 128], F32, tag="wt")
                    nc.gpsimd.tensor_tensor(
                        wt[:], texin[:, 0:128], texin[:, 128:256], op=OP.subtract
                    )
                    wr = cspool.tile([128, 384], F32R, tag="wr")
                    nc.gpsimd.tensor_tensor(
                        wr.rearrange("p (j c) -> p j c", c=3),
                        e.rearrange("p (j c) -> p j c", c=3),
                        wt.unsqueeze(2).broadcast_to([128, 128, 3]),
                        op=OP.mult,
                    )
                    # final per-ray-parity sum into spare cols of ct's bank
                    rp_ = ct[0:2, 128:512]
                    nc.tensor.matmul(rp_, sel2[:], wr[:])
                    outs = cspool.tile([2, 384], F32, tag="outs")
                    nc.vector.tensor_copy(outs[:], rp_)
                    nc.sync.dma_start(out_d[g], outs[:])

    _split_waits(nc, mybir)
    return nc


def _host_prep(origins, directions, t_rand, W0, b0, W1, b1, W2, b2):
    """Build per-core input maps (all numpy, cheap)."""
    f32 = np.float32
    # F-row order: rows 3l+c = sin freq l coord c; 18+3l+c = cos; 36..38 pts
    perm = np.zeros(39, np.int64)
    perm[36:39] = (0, 1, 2)
    for l in range(L):
        for c in range(3):
            perm[3 * l + c] = 3 + 6 * l + c
            perm[18 + 3 * l + c] = 3 + 6 * l + 3 + c
    w0p = np.ascontiguousarray(W0[perm]).astype(np.float16)
    w0rep = np.zeros((128, 256), np.float16)
    w0rep[0:39] = w0p
    w0rep[64:103] = w0p

    w2h = np.empty((128, 8), np.float16)
    w2h[:, 0:4] = W2[0:128].astype(np.float16)
    w2h[:, 4:8] = W2[128:256].astype(np.float16)
    b0t = np.ascontiguousarray(b0.reshape(2, 128).T).astype(f32)
    b1t = np.ascontiguousarray(b1.reshape(2, 128).T).astype(f32)
    b2t = np.broadcast_to(b2.astype(f32), (128, 4)).copy()

    q = np.arange(128)
    rp = q // 64
    s = q % 64
    zcpp = (NEAR + DELTA * s).astype(f32).reshape(128, 1).copy()

    # ltri: cols 0..127 exclusive, 128..255 inclusive
    # ltri[k=(rp',j), m=(rp,s)] = (rp'==rp) & (j < s)  /  (j <= s)
    kk = q
    krp = kk // 64
    kj = kk % 64
    same = (krp[:, None] == rp[None, :])
    ltri = np.zeros((128, 256), f32)
    ltri[:, 0:128] = (same & (kj[:, None] < s[None, :])).astype(f32)
    ltri[:, 128:256] = (same & (kj[:, None] <= s[None, :])).astype(f32)
    sel2 = (krp[:, None] == np.arange(2)[None, :]).astype(f32)
    ident = np.eye(128, dtype=f32)
    identh = np.eye(128, dtype=np.float16)

    # ray_of[J, rp] = 16*(J%128) + 2*(J//128) + rp
    J = np.arange(NBLK)
    ray_of = (16 * (J % 128))[:, None] + (2 * (J // 128))[:, None] + np.arange(2)[None, :]

    in_maps = []
    for core in range(NCORES):
        o = origins[core * BC : (core + 1) * BC].astype(f32)
        d = directions[core * BC : (core + 1) * BC].astype(f32)
        t = t_rand[core * BC : (core + 1) * BC].astype(f32)
        tnat = np.ascontiguousarray(t.reshape(128, 1024))
        # aexp[c, q, J] = o[ray_of[J, rp(q)], c]
        rays_qJ = ray_of[:, :].T[rp]  # [128, NBLK] -> rays_qJ[q, J] = ray_of[J, rp[q]]
        aexp = np.ascontiguousarray(o[rays_qJ].transpose(2, 0, 1))
        bexp = np.ascontiguousarray(d[rays_qJ].transpose(2, 0, 1))
        in_maps.append(
            {
                "tnat": tnat,
                "aexp": aexp,
                "bexp": bexp,
                "w0rep": w0rep,
                "w1": W1.astype(np.float16),
                "w2h": w2h,
                "b0t": b0t,
                "b1t": b1t,
                "b2t": b2t,
                "zcpp": zcpp,
                "ltri": ltri,
                "sel2": sel2,
                "ident": ident,
                "identh": identh,
            }
        )
    return in_maps, ray_of


def kernel(origins, directions, t_rand, W0, b0, W1, b1, W2, b2, near, far,
           **kw):
    assert int(near) == 2 and int(far) == 6
    from concourse.bass_utils import run_bass_kernel_spmd

    if "nc" not in _CACHE:
        _CACHE["nc"] = _build()
    nc = _CACHE["nc"]

    in_maps, ray_of = _host_prep(
        np.asarray(origins), np.asarray(directions), np.asarray(t_rand),
        np.asarray(W0), np.asarray(b0), np.asarray(W1), np.asarray(b1),
        np.asarray(W2), np.asarray(b2),
    )
    res = run_bass_kernel_spmd(
        nc, in_maps, core_ids=list(range(NCORES)), trace=PROFILE
    )
    _CACHE["last_results"] = res
    out = np.empty((B, 3), np.float32)
    for core in range(NCORES):
        oc = res.results[core]["out"].reshape(NGRP, 2, 128, 3)
        # group g holds blocks J = 128*g + i ; ray = 16*i + 2*g + rp
        for g in range(NGRP):
            for rpp in range(2):
                rays = core * BC + 16 * np.arange(128) + 2 * g + rpp
                out[rays] = oc[g, rpp]
    return out


# revision 17
# speedup vs baseline: 1.6475x; 1.0888x over previous
"""NeRF render kernel for 8 Trainium2 NeuronCores.

Data-parallel over rays: core k handles rays [2048*k, 2048*(k+1)).
Per core: positional encoding + 3-layer MLP (39->256->256->4) over
131072 points in feature-major layout (features on partitions, points
on the free dim), then alpha compositing via triangular-matrix matmul
cumulative sums.

Point permutation inside a core: t_rand rows are loaded as
[128 partitions = ray-group i (rays 16i..16i+15), 1024 = (k, s)] and
PE-transposed per 128-column chunk k0 so that partitions become
q = rp*64 + s (rp = ray parity) and columns J = 128*k0 + i denote the
ray pair (16i + 2*k0, 16i + 2*k0 + 1).  All downstream tiles keep that
column order; the host unscatters at the end.

v2 layout: "super-tiles" of 8 blocks (1024 points) processed as two
4-block chunks A/B mapped to partition bands 0:39 / 64:103, so the
feature transposes run col-tiled pairs and the L0 matmuls run
row-tiled pairs (concurrent in the PE array).  PSUM drains are split
ACT/DVE; SBUF-only elementwise work runs on GPSIMD; compositing is
interleaved per group.
"""

import sys
import numpy as np

sys.path.insert(0, "/opt/trn_rl_repo")

S = 64
L = 6
NCORES = 8
B = 16384
BC = B // NCORES          # rays per core
NP = BC * S               # points per core
NBLK = NP // 128          # 1024 ray-pair blocks
NGRP = 8                  # groups of 128 blocks
HB = 64                   # blocks per half-group
NEAR, FAR = 2.0, 6.0
DELTA = (FAR - NEAR) / S
PI = float(np.pi)
TWO_PI = float(2.0 * np.pi)
INV2PI = float(np.float32(1.0 / (2.0 * np.pi)))
MAGIC = 12582912.0  # 1.5 * 2**23: float32 round-to-int trick
C1 = float(np.float32(2.0 * np.pi))
C2 = float(2.0 * np.pi - np.float64(np.float32(2.0 * np.pi)))

_CACHE = {}
PROFILE = False  # test harness sets True to collect an NTFF trace


def _split_waits(nc, mybir):
    """TRN2 allows one sem wait per instruction (two for EventSemaphore);
    this walrus build rejects over-limit instructions, so move excess waits
    onto chained NOPs on the same engine just before the instruction."""
    ctr = 0
    for fn in nc.m.functions:
        for bb in fn.blocks:
            changed = False
            out = []
            for inst in bb.instructions:
                si = inst.sync_info
                cap = 2 if isinstance(inst, mybir.InstEventSemaphore) else 1
                if si is not None and si.on_wait and len(si.on_wait) > cap:
                    waits = list(si.on_wait)
                    for w in waits[:-cap]:
                        nop = mybir.InstNoOp(
                            name=f"wsplit-{ctr}", ins=[], outs=[]
                        )
                        ctr += 1
                        nop.engine = inst.engine
                        nop.sync_info = mybir.SyncInfo(on_wait=[w], on_update=[])
                        nc.register_instruction(nop)
                        out.append(nop)
                    si.on_wait = waits[-cap:]
                    changed = True
                out.append(inst)
            if changed:
                bb.instructions = out
    return ctr


def _build():
    import concourse.bass as bass
    import concourse.mybir as mybir
    import concourse.tile as tile

    dt = mybir.dt
    AF = mybir.ActivationFunctionType
    OP = mybir.AluOpType
    F32 = dt.float32
    F32R = dt.float32r
    F16 = dt.float16

    nc = bass.Bass()

    # ---- DRAM I/O ----
    tnat_d = nc.dram_tensor("tnat", [128, 1024], F32, kind="ExternalInput")
    aexp_d = nc.dram_tensor("aexp", [3, 128, 1024], F32, kind="ExternalInput")
    bexp_d = nc.dram_tensor("bexp", [3, 128, 1024], F32, kind="ExternalInput")
    w0_d = nc.dram_tensor("w0rep", [128, 256], F16, kind="ExternalInput")
    w1_d = nc.dram_tensor("w1", [256, 256], F16, kind="ExternalInput")
    w2_d = nc.dram_tensor("w2h", [128, 8], F16, kind="ExternalInput")
    b0_d = nc.dram_tensor("b0t", [128, 2], F32, kind="ExternalInput")
    b1_d = nc.dram_tensor("b1t", [128, 2], F32, kind="ExternalInput")
    b2_d = nc.dram_tensor("b2t", [128, 4], F32, kind="ExternalInput")
    zcpp_d = nc.dram_tensor("zcpp", [128, 1], F32, kind="ExternalInput")
    ltri_d = nc.dram_tensor("ltri", [128, 256], F32, kind="ExternalInput")
    sel2_d = nc.dram_tensor("sel2", [128, 2], F32R, kind="ExternalInput")
    ident_d = nc.dram_tensor("ident", [128, 128], F32R, kind="ExternalInput")
    identh_d = nc.dram_tensor("identh", [128, 128], F16, kind="ExternalInput")
    out_d = nc.dram_tensor("out", [NGRP, 2, 384], F32, kind="ExternalOutput")

    with tile.TileContext(nc) as tc:
        with (
            tc.tile_pool(name="consts", bufs=1) as cpool,
            tc.tile_pool(name="tall", bufs=1) as tpool,
            tc.tile_pool(name="o2", bufs=2) as o2pool,
        ):
            # ---- load constants / weights ----
            tnat = cpool.tile([128, 1024], F32, tag="tnat")
            nc.sync.dma_start(tnat[:], tnat_d[:])
            w0rep = cpool.tile([128, 256], F16, tag="w0rep")
            nc.sync.dma_start(w0rep[:], w0_d[:])
            w1s0 = cpool.tile([128, 256], F16, tag="w1s0")
            nc.sync.dma_start(w1s0[:], w1_d[0:128, :])
            w1s1 = cpool.tile([128, 256], F16, tag="w1s1")
            nc.sync.dma_start(w1s1[:], w1_d[128:256, :])
            w2s = cpool.tile([128, 8], F16, tag="w2s")
            nc.sync.dma_start(w2s[:], w2_d[:])
            b0t = cpool.tile([128, 2], F32, tag="b0t")
            nc.sync.dma_start(b0t[:], b0_d[:])
            b1t = cpool.tile([128, 2], F32, tag="b1t")
            nc.sync.dma_start(b1t[:], b1_d[:])
            b2t = cpool.tile([128, 4], F32, tag="b2t")
            nc.sync.dma_start(b2t[:], b2_d[:])
            zcpp = cpool.tile([128, 1], F32, tag="zcpp")
            nc.sync.dma_start(zcpp[:], zcpp_d[:])
            ltri = cpool.tile([128, 256], F32, tag="ltri")
            nc.sync.dma_start(ltri[:], ltri_d[:])
            sel2 = cpool.tile([128, 2], F32R, tag="sel2")
            nc.sync.dma_start(sel2[:], sel2_d[:])
            ident = cpool.tile([128, 128], F32R, tag="ident")
            nc.sync.dma_start(ident[:], ident_d[:])
            identh = cpool.tile([128, 128], F16, tag="identh")
            nc.sync.dma_start(identh[:], identh_d[:])
            zerot = cpool.tile([128, 1], F32, tag="zerot")
            nc.vector.memset(zerot[:], 0.0)

            # ---- phase A: t transpose, z, pts (tall/block layout) ----
            # split heavy ops by column halves between DVE and GPSIMD
            zt = tpool.tile([128, 1024], F32, tag="zt")
            pts3 = tpool.tile([128, 3072], F32, tag="pts3")
            pts3r = tpool.tile([128, 3072], F32, tag="pts3r")
            with (
                tc.tile_pool(name="apool", bufs=1) as apool,
                tc.tile_pool(name="ttp", bufs=2, space="PSUM") as ttp_pool,
            ):
                for k0 in range(8):
                    ttp = ttp_pool.tile([128, 128], F32, tag="ttp")
                    nc.tensor.transpose(
                        ttp[:], tnat[:, 128 * k0 : 128 * (k0 + 1)], ident[:].bitcast(F32)
                    )
                    # z = delta * t + zc[s]
                    nc.vector.tensor_scalar(
                        zt[:, 128 * k0 : 128 * (k0 + 1)],
                        ttp[:],
                        DELTA,
                        zcpp[:, 0:1],
                        op0=OP.mult,
                        op1=OP.add,
                    )
                aexs = []
                bexs = []
                for c in range(3):
                    ae = apool.tile([128, 1024], F32, tag=f"aex{c}")
                    nc.sync.dma_start(ae[:], aexp_d[c])
                    be = apool.tile([128, 1024], F32, tag=f"bex{c}")
                    nc.sync.dma_start(be[:], bexp_d[c])
                    aexs.append(ae)
                    bexs.append(be)
                H = 512
                for c in range(3):
                    pv = pts3.rearrange("p (j c) -> p c j", c=3)[:, c, :]
                    for eng, sl in ((nc.vector, slice(0, H)),
                                    (nc.gpsimd, slice(H, 1024))):
                        eng.tensor_tensor(pv[:, sl], zt[:, sl], bexs[c][:, sl], op=OP.mult)
                        eng.tensor_tensor(pv[:, sl], pv[:, sl], aexs[c][:, sl], op=OP.add)
                # base range reduction: pts3r = pts3 - 2pi*round(pts3/2pi)
                kt = apool.tile([128, 3072], F32, tag="kt")
                H3 = 1536
                for eng, sl in ((nc.vector, slice(0, H3)),
                                (nc.gpsimd, slice(H3, 3072))):
                    eng.tensor_scalar(
                        kt[:, sl], pts3[:, sl], INV2PI, MAGIC, op0=OP.mult, op1=OP.add
                    )
                    eng.tensor_scalar(
                        kt[:, sl], kt[:, sl], MAGIC, -C1, op0=OP.subtract, op1=OP.mult
                    )
                    eng.tensor_tensor(
                        pts3r[:, sl], pts3[:, sl], kt[:, sl], op=OP.add
                    )
                    eng.tensor_scalar(
                        pts3r[:, sl], pts3r[:, sl], PI, -PI, op0=OP.min, op1=OP.max
                    )

            # ---- phase B + C: MLP per super-tile, compositing per group ----
            with (
                tc.tile_pool(name="scr", bufs=2) as scrpool,
                tc.tile_pool(name="kpool", bufs=2) as kpool,
                tc.tile_pool(name="sfp", bufs=2) as sfppool,
                tc.tile_pool(name="fs", bufs=2) as fspool,
                tc.tile_pool(name="h0s", bufs=2) as h0spool,
                tc.tile_pool(name="h1s", bufs=2) as h1spool,
                tc.tile_pool(name="cS", bufs=2) as cspool,
                tc.tile_pool(name="tpP", bufs=1, space="PSUM") as tp_pool,
                tc.tile_pool(name="h0P", bufs=2, space="PSUM") as h0_pool,
                tc.tile_pool(name="h1P", bufs=1, space="PSUM") as h1_pool,
                tc.tile_pool(name="oP", bufs=1, space="PSUM") as o_pool,
            ):
                for g in range(NGRP):
                    og = o_pool.tile([128, 512], F32, tag="og")
                    for half in range(2):
                        hg = 2 * g + half
                        # sin-arg chain in (freq-feature, block) layout:
                        # sa2[p, f, j] rows 0:3 = x/2, 3:6 = x, 3(l+1)+c =
                        # wrapped 2^l x.  Cos features come later from the
                        # identity cos(2a) = 1 - 2 sin^2(a), so only sin
                        # args are chained.
                        W = 3 * HB  # cols per feature-triple row-group
                        sa = scrpool.tile([128, 7 * W], F32, tag="sa")
                        # pts views iterated (c outer, j inner) to match
                        # the (f, j) destination order
                        p3t = pts3.rearrange(
                            "p (o j c) -> p o c j", o=16, c=3
                        )[:, hg, :, :]
                        p3rt = pts3r.rearrange(
                            "p (o j c) -> p o c j", o=16, c=3
                        )[:, hg, :, :]
                        nc.vector.tensor_scalar(
                            sa.rearrange("p (f c j) -> p f c j", f=7, c=3)[:, 0],
                            p3rt, 0.5, None, op0=OP.mult,
                        )
                        nc.vector.tensor_copy(
                            sa.rearrange("p (f c j) -> p f c j", f=7, c=3)[:, 1],
                            p3rt,
                        )
                        # r_l = 2 r_{l-1} - 2pi*round(r_{l-1}/pi):
                        # k-scale on DVE (contiguous, 2x mode), the
                        # tensor_tensor on gpsimd, double on DVE
                        for l in range(1, L):
                            prev = sa[:, l * W : (l + 1) * W]
                            cur = sa[:, (l + 1) * W : (l + 2) * W]
                            kb = kpool.tile([128, W], F32, tag="kb")
                            nc.vector.tensor_scalar(
                                kb[:], prev, 2.0 * INV2PI, MAGIC,
                                op0=OP.mult, op1=OP.add,
                            )
                            nc.vector.tensor_scalar(
                                kb[:], kb[:], MAGIC, -PI,
                                op0=OP.subtract, op1=OP.mult,
                            )
                            nc.gpsimd.tensor_tensor(cur, prev, kb[:], op=OP.add)
                            nc.vector.tensor_scalar(
                                cur, cur, 2.0, None, op0=OP.mult
                            )
                        # staging tile (j, f64): per block j a 64-slot group
                        # [0:18 sin, 18:36 cos, 36:39 raw, 39:42 sin(x/2),
                        # 42:64 pad].  This layout makes each 128-col pair
                        # of blocks a contiguous [128, 128] unit that a
                        # REGULAR matmul (identity moving, data stationary,
                        # FWL-eligible) transposes into 64-aligned bands —
                        # and regular MMs keep the HAM clock warm.
                        sf = sfppool.tile([128, 64 * HB], F16, tag="sf")
                        sfv = sf.rearrange("p (j f) -> p j f", f=64)
                        # gathered-read sins (iterate j outer, f inner)
                        saj = sa.rearrange("p (f j) -> p j f", j=HB)
                        nc.scalar.activation(
                            sfv[:, :, 0:18], saj[:, :, 3:21], AF.Sin
                        )
                        nc.scalar.activation(
                            sfv[:, :, 39:42], saj[:, :, 0:3], AF.Sin
                        )
                        # cos_l = 1 - 2 sin(2^{l-1} x)^2   (fp16, DVE)
                        for l in range(L):
                            src = (sfv[:, :, 39:42] if l == 0
                                   else sfv[:, :, 3 * (l - 1) : 3 * l])
                            kb16 = kpool.tile([128, W], F16, tag="kb16")
                            kb16v = kb16.rearrange("p (j c) -> p j c", c=3)
                            nc.vector.tensor_tensor(kb16v, src, src, op=OP.mult)
                            nc.vector.tensor_scalar(
                                sfv[:, :, 18 + 3 * l : 21 + 3 * l],
                                kb16v, -2.0, 1.0, op0=OP.mult, op1=OP.add,
                            )
                        nc.vector.tensor_copy(
                            sfv[:, :, 36:39],
                            pts3.rearrange("p (j c) -> p j c", c=3)[
                                :, HB * hg : HB * (hg + 1), :
                            ],
                        )
                        # super-tiles: 8 blocks = 4 contiguous 2-block
                        # chunks; each chunk is transposed by a REGULAR
                        # matmul (chunk stationary + FWL, identity moving)
                        # landing block-parity bands at partitions 0:64 /
                        # 64:128 directly — and counting as PE activity
                        # so the HAM clock stays warm.
                        for st in range(HB // 8):
                            tp = tp_pool.tile([128, 512], F32, tag="tp")
                            for k in range(4):
                                jj = 4 * st + k
                                nc.tensor.matmul(
                                    tp[:, 128 * k : 128 * (k + 1)],
                                    sf[:, 128 * jj : 128 * (jj + 1)],
                                    identh[:],
                                )
                            # fp32 PSUM -> fp16 SBUF staging; alternate
                            fs = fspool.tile([128, 512], F16, tag="fs")
                            if st % 2 == 0:
                                nc.scalar.activation(
                                    fs[:], tp[:], AF.Copy
                                )
                            else:
                                nc.vector.tensor_copy(fs[:], tp[:])
                            # L0: row-tiled pairs (bands 0 / 64); PSUM tile
                            # per hidden-half holds [chunk A | chunk B] so
                            # the drain is one FD-1024 op with uniform bias
                            h0ss = []
                            for h in range(2):
                                h0p = h0_pool.tile([128, 1024], F32, tag="h0p")
                                for x in range(2):
                                    lo = 64 * x
                                    nc.tensor.matmul(
                                        h0p[:, 512 * x : 512 * (x + 1)],
                                        w0rep[lo : lo + 39, 128 * h : 128 * (h + 1)],
                                        fs[lo : lo + 39, :],
                                    )
                                h0s = h0spool.tile([128, 1024], F16, tag="h0s")
                                if h == 0:
                                    nc.scalar.activation(
                                        h0s[:], h0p[:], AF.Relu,
                                        bias=b0t[:, 0:1],
                                    )
                                else:
                                    nc.vector.tensor_scalar(
                                        h0s[:], h0p[:],
                                        b0t[:, 1:2], 0.0, op0=OP.add, op1=OP.max,
                                    )
                                h0ss.append(h0s)
                            # L1: PSUM tile per out-half g = [A | B]
                            h1ss = []
                            for gh in range(2):
                                h1p = h1_pool.tile([128, 1024], F32, tag="h1p")
                                for x in range(2):
                                    nc.tensor.matmul(
                                        h1p[:, 512 * x : 512 * (x + 1)],
                                        w1s0[:, 128 * gh : 128 * (gh + 1)],
                                        h0ss[0][:, 512 * x : 512 * (x + 1)],
                                        start=True,
                                        stop=False,
                                    )
                                    nc.tensor.matmul(
                                        h1p[:, 512 * x : 512 * (x + 1)],
                                        w1s1[:, 128 * gh : 128 * (gh + 1)],
                                        h0ss[1][:, 512 * x : 512 * (x + 1)],
                                        start=False,
                                        stop=True,
                                    )
                                h1s = h1spool.tile([128, 1024], F16, tag="h1s")
                                if gh == 0:
                                    nc.scalar.activation(
                                        h1s[:], h1p[:], AF.Relu,
                                        bias=b1t[:, 0:1],
                                    )
                                else:
                                    nc.vector.tensor_scalar(
                                        h1s[:], h1p[:],
                                        b1t[:, 1:2], 0.0,
                                        op0=OP.add, op1=OP.max,
                                    )
                                h1ss.append(h1s)
                            # L2: activations stationary, W2 moving
                            # (band x holds blocks of parity x)
                            for x in range(2):
                                for jp in range(4):
                                    jj = HB * half + 8 * st + 2 * jp + x
                                    nc.tensor.matmul(
                                        og[:, 4 * jj : 4 * (jj + 1)],
                                        h1ss[0][:, 512 * x + 128 * jp : 512 * x + 128 * (jp + 1)],
                                        w2s[:, 0:4],
                                        start=True,
                                        stop=False,
                                    )
                                    nc.tensor.matmul(
                                        og[:, 4 * jj : 4 * (jj + 1)],
                                        h1ss[1][:, 512 * x + 128 * jp : 512 * x + 128 * (jp + 1)],
                                        w2s[:, 4:8],
                                        start=False,
                                        stop=True,
                                    )
                    # ---- drain O psum -> O2 sbuf (per-channel bias ops,
                    # relu on sigma; split ACT/DVE) ----
                    o2 = o2pool.tile([128, 512], F32, tag="o2")
                    ogv = og.rearrange("p (j c) -> p j c", c=4)
                    o2v = o2.rearrange("p (j c) -> p j c", c=4)
                    nc.scalar.activation(
                        o2v[:, :, 0], ogv[:, :, 0], AF.Identity, bias=b2t[:, 0:1]
                    )
                    nc.vector.tensor_scalar(
                        o2v[:, :, 1], ogv[:, :, 1], b2t[:, 1:2], None, op0=OP.add
                    )
                    nc.scalar.activation(
                        o2v[:, :, 2], ogv[:, :, 2], AF.Identity, bias=b2t[:, 2:3]
                    )
                    nc.vector.tensor_scalar(
                        o2v[:, :, 3], ogv[:, :, 3], b2t[:, 3:4], 0.0,
                        op0=OP.add, op1=OP.max,
                    )
                    # ---- compositing for this group (interleaved) ----
                    e = cspool.tile([128, 384], F32, tag="e")
                    nc.scalar.activation(
                        e.rearrange("p (j c) -> p j c", c=3),
                        o2v[:, :, 0:3],
                        AF.Sigmoid,
                    )
                    # scans: exclusive & inclusive cumsum of sigma over s
                    # (ct borrows the h1 PSUM ring)
                    ct = h1_pool.tile([128, 1024], F32, tag="h1p")
                    sig = o2v[:, :, 3]
                    nc.tensor.matmul(ct[:, 0:128], ltri[:, 0:128], sig)
                    nc.tensor.matmul(ct[:, 128:256], ltri[:, 128:256], sig)
                    texin = cspool.tile([128, 256], F32, tag="texin")
                    nc.scalar.activation(texin[:], ct[:, 0:256], AF.Exp, scale=-DELTA)
                    wt = cspool.tile([128, 128], F32, tag="wt")
                    nc.gpsimd.tensor_tensor(
                        wt[:], texin[:, 0:128], texin[:, 128:256], op=OP.subtract
                    )
                    wr = cspool.tile([128, 384], F32R, tag="wr")
                    nc.gpsimd.tensor_tensor(
                        wr.rearrange("p (j c) -> p j c", c=3),
                        e.rearrange("p (j c) -> p j c", c=3),
                        wt.unsqueeze(2).broadcast_to([128, 128, 3]),
                        op=OP.mult,
                    )
                    # final per-ray-parity sum into spare cols of ct's bank
                    rp_ = ct[0:2, 128:512]
                    nc.tensor.matmul(rp_, sel2[:], wr[:])
                    outs = cspool.tile([2, 384], F32, tag="outs")
                    nc.vector.tensor_copy(outs[:], rp_)
                    nc.sync.dma_start(out_d[g], outs[:])

    _split_waits(nc, mybir)
    return nc


def _host_prep(origins, directions, t_rand, W0, b0, W1, b1, W2, b2):
    """Build per-core input maps (all numpy, cheap)."""
    f32 = np.float32
    # F-row order: rows 3l+c = sin freq l coord c; 18+3l+c = cos; 36..38 pts
    perm = np.zeros(39, np.int64)
    perm[36:39] = (0, 1, 2)
    for l in range(L):
        for c in range(3):
            perm[3 * l + c] = 3 + 6 * l + c
            perm[18 + 3 * l + c] = 3 + 6 * l + 3 + c
    w0p = np.ascontiguousarray(W0[perm]).astype(np.float16)
    w0rep = np.zeros((128, 256), np.float16)
    w0rep[0:39] = w0p
    w0rep[64:103] = w0p

    w2h = np.empty((128, 8), np.float16)
    w2h[:, 0:4] = W2[0:128].astype(np.float16)
    w2h[:, 4:8] = W2[128:256].astype(np.float16)
    b0t = np.ascontiguousarray(b0.reshape(2, 128).T).astype(f32)
    b1t = np.ascontiguousarray(b1.reshape(2, 128).T).astype(f32)
    b2t = np.broadcast_to(b2.astype(f32), (128, 4)).copy()

    q = np.arange(128)
    rp = q // 64
    s = q % 64
    zcpp = (NEAR + DELTA * s).astype(f32).reshape(128, 1).copy()

    # ltri: cols 0..127 exclusive, 128..255 inclusive
    # ltri[k=(rp',j), m=(rp,s)] = (rp'==rp) & (j < s)  /  (j <= s)
    kk = q
    krp = kk // 64
    kj = kk % 64
    same = (krp[:, None] == rp[None, :])
    ltri = np.zeros((128, 256), f32)
    ltri[:, 0:128] = (same & (kj[:, None] < s[None, :])).astype(f32)
    ltri[:, 128:256] = (same & (kj[:, None] <= s[None, :])).astype(f32)
    sel2 = (krp[:, None] == np.arange(2)[None, :]).astype(f32)
    ident = np.eye(128, dtype=f32)
    identh = np.eye(128, dtype=np.float16)

    # ray_of[J, rp] = 16*(J%128) + 2*(J//128) + rp
    J = np.arange(NBLK)
    ray_of = (16 * (J % 128))[:, None] + (2 * (J // 128))[:, None] + np.arange(2)[None, :]

    in_maps = []
    for core in range(NCORES):
        o = origins[core * BC : (core + 1) * BC].astype(f32)
        d = directions[core * BC : (core + 1) * BC].astype(f32)
        t = t_rand[core * BC : (core + 1) * BC].astype(f32)
        tnat = np.ascontiguousarray(t.reshape(128, 1024))
        # aexp[c, q, J] = o[ray_of[J, rp(q)], c]
        rays_qJ = ray_of[:, :].T[rp]  # [128, NBLK] -> rays_qJ[q, J] = ray_of[J, rp[q]]
        aexp = np.ascontiguousarray(o[rays_qJ].transpose(2, 0, 1))
        bexp = np.ascontiguousarray(d[rays_qJ].transpose(2, 0, 1))
        in_maps.append(
            {
                "tnat": tnat,
                "aexp": aexp,
                "bexp": bexp,
                "w0rep": w0rep,
                "w1": W1.astype(np.float16),
                "w2h": w2h,
                "b0t": b0t,
                "b1t": b1t,
                "b2t": b2t,
                "zcpp": zcpp,
                "ltri": ltri,
                "sel2": sel2,
                "ident": ident,
                "identh": identh,
            }
        )
    return in_maps, ray_of


def kernel(origins, directions, t_rand, W0, b0, W1, b1, W2, b2, near, far,
           **kw):
    assert int(near) == 2 and int(far) == 6
    from concourse.bass_utils import run_bass_kernel_spmd

    if "nc" not in _CACHE:
        _CACHE["nc"] = _build()
    nc = _CACHE["nc"]

    in_maps, ray_of = _host_prep(
        np.asarray(origins), np.asarray(directions), np.asarray(t_rand),
        np.asarray(W0), np.asarray(b0), np.asarray(W1), np.asarray(b1),
        np.asarray(W2), np.asarray(b2),
    )
    res = run_bass_kernel_spmd(
        nc, in_maps, core_ids=list(range(NCORES)), trace=PROFILE
    )
    _CACHE["last_results"] = res
    out = np.empty((B, 3), np.float32)
    for core in range(NCORES):
        oc = res.results[core]["out"].reshape(NGRP, 2, 128, 3)
        # group g holds blocks J = 128*g + i ; ray = 16*i + 2*g + rp
        for g in range(NGRP):
            for rpp in range(2):
                rays = core * BC + 16 * np.arange(128) + 2 * g + rpp
                out[rays] = oc[g, rpp]
    return out


# revision 20
# speedup vs baseline: 2.5939x; 1.5744x over previous
"""NeRF render kernel for 8 Trainium2 NeuronCores.

Data-parallel over rays: core k handles rays [2048*k, 2048*(k+1)).
Per core: positional encoding + 3-layer MLP (39->256->256->4) over
131072 points in feature-major layout (features on partitions, points
on the free dim), then alpha compositing via triangular-matrix matmul
cumulative sums.

Point permutation inside a core: t_rand rows are loaded as
[128 partitions = ray-group i (rays 16i..16i+15), 1024 = (k, s)] and
PE-transposed per 128-column chunk k0 so that partitions become
q = rp*64 + s (rp = ray parity) and columns J = 128*k0 + i denote the
ray pair (16i + 2*k0, 16i + 2*k0 + 1).  All downstream tiles keep that
column order; the host unscatters at the end.

v2 layout: "super-tiles" of 8 blocks (1024 points) processed as two
4-block chunks A/B mapped to partition bands 0:39 / 64:103, so the
feature transposes run col-tiled pairs and the L0 matmuls run
row-tiled pairs (concurrent in the PE array).  PSUM drains are split
ACT/DVE; SBUF-only elementwise work runs on GPSIMD; compositing is
interleaved per group.
"""

import sys
import numpy as np

sys.path.insert(0, "/opt/trn_rl_repo")

S = 64
L = 6
NCORES = 8
B = 16384
BC = B // NCORES          # rays per core
NP = BC * S               # points per core
NBLK = NP // 128          # 1024 ray-pair blocks
NGRP = 8                  # groups of 128 blocks
HB = 64                   # blocks per half-group
NEAR, FAR = 2.0, 6.0
DELTA = (FAR - NEAR) / S
PI = float(np.pi)
TWO_PI = float(2.0 * np.pi)
INV2PI = float(np.float32(1.0 / (2.0 * np.pi)))
MAGIC = 12582912.0  # 1.5 * 2**23: float32 round-to-int trick
C1 = float(np.float32(2.0 * np.pi))
C2 = float(2.0 * np.pi - np.float64(np.float32(2.0 * np.pi)))

_CACHE = {}
PROFILE = False  # test harness sets True to collect an NTFF trace


def _split_waits(nc, mybir):
    """TRN2 allows one sem wait per instruction (two for EventSemaphore);
    this walrus build rejects over-limit instructions, so move excess waits
    onto chained NOPs on the same engine just before the instruction."""
    ctr = 0
    for fn in nc.m.functions:
        for bb in fn.blocks:
            changed = False
            out = []
            for inst in bb.instructions:
                si = inst.sync_info
                cap = 2 if isinstance(inst, mybir.InstEventSemaphore) else 1
                if si is not None and si.on_wait and len(si.on_wait) > cap:
                    waits = list(si.on_wait)
                    for w in waits[:-cap]:
                        nop = mybir.InstNoOp(
                            name=f"wsplit-{ctr}", ins=[], outs=[]
                        )
                        ctr += 1
                        nop.engine = inst.engine
                        nop.sync_info = mybir.SyncInfo(on_wait=[w], on_update=[])
                        nc.register_instruction(nop)
                        out.append(nop)
                    si.on_wait = waits[-cap:]
                    changed = True
                out.append(inst)
            if changed:
                bb.instructions = out
    return ctr


def _build():
    import concourse.bass as bass
    import concourse.mybir as mybir
    import concourse.tile as tile

    dt = mybir.dt
    AF = mybir.ActivationFunctionType
    OP = mybir.AluOpType
    F32 = dt.float32
    F32R = dt.float32r
    F16 = dt.float16

    nc = bass.Bass()

    # ---- DRAM I/O ----
    tnat_d = nc.dram_tensor("tnat", [128, 1024], F32, kind="ExternalInput")
    aexp_d = nc.dram_tensor("aexp", [3, 128, 1024], F32, kind="ExternalInput")
    bexp_d = nc.dram_tensor("bexp", [3, 128, 1024], F32, kind="ExternalInput")
    w0_d = nc.dram_tensor("w0rep", [128, 256], F16, kind="ExternalInput")
    w1_d = nc.dram_tensor("w1", [256, 256], F16, kind="ExternalInput")
    w2_d = nc.dram_tensor("w2h", [128, 8], F16, kind="ExternalInput")
    b0_d = nc.dram_tensor("b0t", [128, 2], F32, kind="ExternalInput")
    b1_d = nc.dram_tensor("b1t", [128, 2], F32, kind="ExternalInput")
    b2_d = nc.dram_tensor("b2t", [128, 4], F32, kind="ExternalInput")
    zcpp_d = nc.dram_tensor("zcpp", [128, 1], F32, kind="ExternalInput")
    ltri_d = nc.dram_tensor("ltri", [128, 256], F32, kind="ExternalInput")
    sel2_d = nc.dram_tensor("sel2", [128, 2], F32R, kind="ExternalInput")
    ident_d = nc.dram_tensor("ident", [128, 128], F32R, kind="ExternalInput")
    identh_d = nc.dram_tensor("identh", [128, 128], F16, kind="ExternalInput")
    out_d = nc.dram_tensor("out", [NGRP, 2, 384], F32, kind="ExternalOutput")

    with tile.TileContext(nc) as tc:
        with (
            tc.tile_pool(name="consts", bufs=1) as cpool,
            tc.tile_pool(name="tall", bufs=1) as tpool,
            tc.tile_pool(name="o2", bufs=2) as o2pool,
        ):
            # ---- load constants / weights ----
            tnat = cpool.tile([128, 1024], F32, tag="tnat")
            nc.sync.dma_start(tnat[:], tnat_d[:])
            w0rep = cpool.tile([128, 256], F16, tag="w0rep")
            nc.sync.dma_start(w0rep[:], w0_d[:])
            w1s0 = cpool.tile([128, 256], F16, tag="w1s0")
            nc.sync.dma_start(w1s0[:], w1_d[0:128, :])
            w1s1 = cpool.tile([128, 256], F16, tag="w1s1")
            nc.sync.dma_start(w1s1[:], w1_d[128:256, :])
            w2s = cpool.tile([128, 8], F16, tag="w2s")
            nc.sync.dma_start(w2s[:], w2_d[:])
            b0t = cpool.tile([128, 2], F32, tag="b0t")
            nc.sync.dma_start(b0t[:], b0_d[:])
            b1t = cpool.tile([128, 2], F32, tag="b1t")
            nc.sync.dma_start(b1t[:], b1_d[:])
            b2t = cpool.tile([128, 4], F32, tag="b2t")
            nc.sync.dma_start(b2t[:], b2_d[:])
            zcpp = cpool.tile([128, 1], F32, tag="zcpp")
            nc.sync.dma_start(zcpp[:], zcpp_d[:])
            ltri = cpool.tile([128, 256], F32, tag="ltri")
            nc.sync.dma_start(ltri[:], ltri_d[:])
            sel2 = cpool.tile([128, 2], F32R, tag="sel2")
            nc.sync.dma_start(sel2[:], sel2_d[:])
            ident = cpool.tile([128, 128], F32R, tag="ident")
            nc.sync.dma_start(ident[:], ident_d[:])
            identh = cpool.tile([128, 128], F16, tag="identh")
            nc.sync.dma_start(identh[:], identh_d[:])
            zerot = cpool.tile([128, 1], F32, tag="zerot")
            nc.vector.memset(zerot[:], 0.0)

            # ---- phase A: t transpose, z, pts (tall/block layout) ----
            # split heavy ops by column halves between DVE and GPSIMD
            zt = tpool.tile([128, 1024], F32, tag="zt")
            pts3 = tpool.tile([128, 3072], F32, tag="pts3")
            pts3r = tpool.tile([128, 3072], F32, tag="pts3r")
            with (
                tc.tile_pool(name="apool", bufs=1) as apool,
                tc.tile_pool(name="ttp", bufs=2, space="PSUM") as ttp_pool,
            ):
                for k0 in range(8):
                    ttp = ttp_pool.tile([128, 128], F32, tag="ttp")
                    nc.tensor.transpose(
                        ttp[:], tnat[:, 128 * k0 : 128 * (k0 + 1)], ident[:].bitcast(F32)
                    )
                    # z = delta * t + zc[s]
                    nc.vector.tensor_scalar(
                        zt[:, 128 * k0 : 128 * (k0 + 1)],
                        ttp[:],
                        DELTA,
                        zcpp[:, 0:1],
                        op0=OP.mult,
                        op1=OP.add,
                    )
                aexs = []
                bexs = []
                for c in range(3):
                    ae = apool.tile([128, 1024], F32, tag=f"aex{c}")
                    nc.sync.dma_start(ae[:], aexp_d[c])
                    be = apool.tile([128, 1024], F32, tag=f"bex{c}")
                    nc.sync.dma_start(be[:], bexp_d[c])
                    aexs.append(ae)
                    bexs.append(be)
                H = 512
                for c in range(3):
                    pv = pts3.rearrange("p (j c) -> p c j", c=3)[:, c, :]
                    for eng, sl in ((nc.vector, slice(0, H)),
                                    (nc.gpsimd, slice(H, 1024))):
                        eng.tensor_tensor(pv[:, sl], zt[:, sl], bexs[c][:, sl], op=OP.mult)
                        eng.tensor_tensor(pv[:, sl], pv[:, sl], aexs[c][:, sl], op=OP.add)
                # base range reduction: pts3r = pts3 - 2pi*round(pts3/2pi)
                kt = apool.tile([128, 3072], F32, tag="kt")
                H3 = 1536
                for eng, sl in ((nc.vector, slice(0, H3)),
                                (nc.gpsimd, slice(H3, 3072))):
                    eng.tensor_scalar(
                        kt[:, sl], pts3[:, sl], INV2PI, MAGIC, op0=OP.mult, op1=OP.add
                    )
                    eng.tensor_scalar(
                        kt[:, sl], kt[:, sl], MAGIC, -C1, op0=OP.subtract, op1=OP.mult
                    )
                    eng.tensor_tensor(
                        pts3r[:, sl], pts3[:, sl], kt[:, sl], op=OP.add
                    )
                    eng.tensor_scalar(
                        pts3r[:, sl], pts3r[:, sl], PI, -PI, op0=OP.min, op1=OP.max
                    )

            # ---- phase B + C: software-pipelined over 128 super-tiles ----
            # At iteration it the PE runs transposes(it), L0(it-1),
            # L1(it-2), L2(it-3): every stage consumes activations that
            # were drained a full iteration earlier, so the PE never
            # stalls on ACT/DVE drains and the HAM clock stays warm.
            NS = NBLK // 8            # 128 supers; super s = blocks 8s..8s+7
            NHG = 2 * NGRP            # 16 half-groups of 8 supers
            W = 3 * HB
            with (
                tc.tile_pool(name="scr", bufs=2) as scrpool,
                tc.tile_pool(name="kpool", bufs=2) as kpool,
                tc.tile_pool(name="sfp", bufs=2) as sfppool,
                tc.tile_pool(name="fs", bufs=3) as fspool,
                tc.tile_pool(name="h0s", bufs=3) as h0spool,
                tc.tile_pool(name="h1s", bufs=3) as h1spool,
                tc.tile_pool(name="cS", bufs=2) as cspool,
                tc.tile_pool(name="tpP", bufs=1, space="PSUM") as tp_pool,
                tc.tile_pool(name="h0P", bufs=3, space="PSUM") as h0_pool,
                tc.tile_pool(name="h1P", bufs=3, space="PSUM") as h1_pool,
                tc.tile_pool(name="oP", bufs=1, space="PSUM") as o_pool,
            ):
                sf_t = {}
                fs_t = {}
                h0_t = {}
                h1_t = {}
                og_t = {}

                def chain_gen(hg):
                    """Sin-arg chain + staging for one half-group, split
                    into 8 steps (one per pipeline iteration).

                    sa (f, j): rows 0:3 = x/2, 3:6 = x, 3(l+1)+c = wrapped
                    2^l x.  sf (j, f64): per block j a 64-slot group
                    [0:18 sin, 18:36 cos, 36:39 raw, 39:42 sin(x/2),
                    42:64 pad] so each 2-block pair is a contiguous
                    [128, 128] unit that a REGULAR matmul (chunk
                    stationary + FWL, identity moving) transposes into
                    64-aligned bands -- counting as PE activity (warm HAM).
                    """
                    sa = scrpool.tile([128, 7 * W], F32, tag="sa", name="sa")
                    p3rt = pts3r.rearrange(
                        "p (o j c) -> p o c j", o=NHG, c=3
                    )[:, hg, :, :]
                    sa7 = sa.rearrange("p (f c j) -> p f c j", f=7, c=3)
                    nc.vector.tensor_scalar(
                        sa7[:, 0], p3rt, 0.5, None, op0=OP.mult
                    )
                    nc.vector.tensor_copy(sa7[:, 1], p3rt)
                    yield
                    # r_l = 2 r_{l-1} - 2pi*round(r_{l-1}/pi)
                    for l in range(1, L):
                        prev = sa[:, l * W : (l + 1) * W]
                        cur = sa[:, (l + 1) * W : (l + 2) * W]
                        kb = kpool.tile([128, W], F32, tag="kb", name="kb")
                        nc.vector.tensor_scalar(
                            kb[:], prev, 2.0 * INV2PI, MAGIC,
                            op0=OP.mult, op1=OP.add,
                        )
                        nc.vector.tensor_scalar(
                            kb[:], kb[:], MAGIC, -PI,
                            op0=OP.subtract, op1=OP.mult,
                        )
                        nc.gpsimd.tensor_tensor(cur, prev, kb[:], op=OP.add)
                        nc.vector.tensor_scalar(
                            cur, cur, 2.0, None, op0=OP.mult
                        )
                        if l < L - 1:
                            yield
                    sf = sfppool.tile([128, 64 * HB], F16, tag="sf", name="sf")
                    sf_t[hg] = sf
                    sfv = sf.rearrange("p (j f) -> p j f", f=64)
                    saj = sa.rearrange("p (f j) -> p j f", j=HB)
                    nc.scalar.activation(
                        sfv[:, :, 39:42], saj[:, :, 0:3], AF.Sin
                    )
                    yield
                    nc.scalar.activation(
                        sfv[:, :, 0:18], saj[:, :, 3:21], AF.Sin
                    )
                    # cos_l = 1 - 2 sin(2^{l-1} x)^2   (fp16, DVE)
                    for l in range(L):
                        if l == 2:
                            yield
                        src = (sfv[:, :, 39:42] if l == 0
                               else sfv[:, :, 3 * (l - 1) : 3 * l])
                        kb16 = kpool.tile([128, W], F16, tag="kb16", name="kb16")
                        kb16v = kb16.rearrange("p (j c) -> p j c", c=3)
                        nc.vector.tensor_tensor(kb16v, src, src, op=OP.mult)
                        nc.vector.tensor_scalar(
                            sfv[:, :, 18 + 3 * l : 21 + 3 * l],
                            kb16v, -2.0, 1.0, op0=OP.mult, op1=OP.add,
                        )
                    nc.vector.tensor_copy(
                        sfv[:, :, 36:39],
                        pts3.rearrange("p (j c) -> p j c", c=3)[
                            :, HB * hg : HB * (hg + 1), :
                        ],
                    )
                    yield

                def stage_T(s):
                    sf = sf_t[s // 8]
                    stl = s % 8
                    tp = tp_pool.tile([128, 512], F32, tag="tp", name="tp")
                    for k in range(4):
                        jj = 4 * stl + k
                        nc.tensor.matmul(
                            tp[:, 128 * k : 128 * (k + 1)],
                            sf[:, 128 * jj : 128 * (jj + 1)],
                            identh[:],
                        )
                    fs = fspool.tile([128, 512], F16, tag="fs", name="fs")
                    if s % 2 == 0:
                        nc.scalar.activation(fs[:], tp[:], AF.Copy)
                    else:
                        nc.vector.tensor_copy(fs[:], tp[:])
                    fs_t[s] = fs

                def stage_L0(s):
                    # L0 row-tiled band pairs; drain each [128, 512] PSUM
                    # tile right after its matmul (2 ACT + 2 DVE per super)
                    fs = fs_t.pop(s)
                    h0ss = [
                        h0spool.tile([128, 1024], F16, tag="h0s", name=f"h0s{s}_{h}")
                        for h in range(2)
                    ]
                    for h in range(2):
                        for x in range(2):
                            h0p = h0_pool.tile([128, 512], F32, tag="h0p", name="h0p")
                            lo = 64 * x
                            nc.tensor.matmul(
                                h0p[:],
                                w0rep[lo : lo + 39, 128 * h : 128 * (h + 1)],
                                fs[lo : lo + 39, :],
                            )
                            dst = h0ss[h][:, 512 * x : 512 * (x + 1)]
                            if h == 0:
                                nc.scalar.activation(
                                    dst, h0p[:], AF.Relu, bias=b0t[:, 0:1]
                                )
                            else:
                                nc.vector.tensor_scalar(
                                    dst, h0p[:], b0t[:, 1:2], 0.0,
                                    op0=OP.add, op1=OP.max,
                                )
                    h0_t[s] = h0ss

                def stage_L1(s):
                    h0ss = h0_t.pop(s)
                    h1ss = [
                        h1spool.tile([128, 1024], F16, tag="h1s", name=f"h1s{s}_{g}")
                        for g in range(2)
                    ]
                    for gh in range(2):
                        for x in range(2):
                            h1p = h1_pool.tile([128, 512], F32, tag="h1p", name="h1p")
                            nc.tensor.matmul(
                                h1p[:],
                                w1s0[:, 128 * gh : 128 * (gh + 1)],
                                h0ss[0][:, 512 * x : 512 * (x + 1)],
                                start=True,
                                stop=False,
                            )
                            nc.tensor.matmul(
                                h1p[:],
                                w1s1[:, 128 * gh : 128 * (gh + 1)],
                                h0ss[1][:, 512 * x : 512 * (x + 1)],
                                start=False,
                                stop=True,
                            )
                            dst = h1ss[gh][:, 512 * x : 512 * (x + 1)]
                            if gh == 0:
                                nc.scalar.activation(
                                    dst, h1p[:], AF.Relu, bias=b1t[:, 0:1]
                                )
                            else:
                                nc.vector.tensor_scalar(
                                    dst, h1p[:], b1t[:, 1:2], 0.0,
                                    op0=OP.add, op1=OP.max,
                                )
                    h1_t[s] = h1ss

                def stage_L2(s):
                    h1ss = h1_t.pop(s)
                    g = s // 16
                    if s % 16 == 0:
                        og_t[g] = o_pool.tile([128, 512], F32, tag="og", name="og")
                    og = og_t[g]
                    # band x holds blocks of parity x
                    for x in range(2):
                        for jp in range(4):
                            jj = 8 * (s % 16) + 2 * jp + x
                            nc.tensor.matmul(
                                og[:, 4 * jj : 4 * (jj + 1)],
                                h1ss[0][:, 512 * x + 128 * jp : 512 * x + 128 * (jp + 1)],
                                w2s[:, 0:4],
                                start=True,
                                stop=False,
                            )
                            nc.tensor.matmul(
                                og[:, 4 * jj : 4 * (jj + 1)],
                                h1ss[1][:, 512 * x + 128 * jp : 512 * x + 128 * (jp + 1)],
                                w2s[:, 4:8],
                                start=False,
                                stop=True,
                            )
                    if s % 16 == 15:
                        emit_groupC(g)

                def emit_groupC(g):
                    # og -> o2 (per-channel bias, relu on sigma), then
                    # compositing; ct borrows the h1 PSUM ring
                    og = og_t.pop(g)
                    o2 = o2pool.tile([128, 512], F32, tag="o2", name="o2")
                    ogv = og.rearrange("p (j c) -> p j c", c=4)
                    o2v = o2.rearrange("p (j c) -> p j c", c=4)
                    nc.scalar.activation(
                        o2v[:, :, 0], ogv[:, :, 0], AF.Identity, bias=b2t[:, 0:1]
                    )
                    nc.vector.tensor_scalar(
                        o2v[:, :, 1], ogv[:, :, 1], b2t[:, 1:2], None, op0=OP.add
                    )
                    nc.scalar.activation(
                        o2v[:, :, 2], ogv[:, :, 2], AF.Identity, bias=b2t[:, 2:3]
                    )
                    nc.vector.tensor_scalar(
                        o2v[:, :, 3], ogv[:, :, 3], b2t[:, 3:4], 0.0,
                        op0=OP.add, op1=OP.max,
                    )
                    e = cspool.tile([128, 384], F32, tag="e", name="e")
                    nc.scalar.activation(
                        e.rearrange("p (j c) -> p j c", c=3),
                        o2v[:, :, 0:3],
                        AF.Sigmoid,
                    )
                    # scans: exclusive & inclusive cumsum of sigma over s
                    ct = h1_pool.tile([128, 512], F32, tag="h1p", name="ct")
                    sig = o2v[:, :, 3]
                    nc.tensor.matmul(ct[:, 0:128], ltri[:, 0:128], sig)
                    nc.tensor.matmul(ct[:, 128:256], ltri[:, 128:256], sig)
                    texin = cspool.tile([128, 256], F32, tag="texin", name="texin")
                    nc.scalar.activation(texin[:], ct[:, 0:256], AF.Exp, scale=-DELTA)
                    wt = cspool.tile([128, 128], F32, tag="wt", name="wt")
                    nc.gpsimd.tensor_tensor(
                        wt[:], texin[:, 0:128], texin[:, 128:256], op=OP.subtract
                    )
                    wr = cspool.tile([128, 384], F32R, tag="wr", name="wr")
                    nc.gpsimd.tensor_tensor(
                        wr.rearrange("p (j c) -> p j c", c=3),
                        e.rearrange("p (j c) -> p j c", c=3),
                        wt.unsqueeze(2).broadcast_to([128, 128, 3]),
                        op=OP.mult,
                    )
                    # final per-ray-parity sum into spare cols of ct's bank
                    rp_ = ct[0:2, 128:512]
                    nc.tensor.matmul(rp_, sel2[:], wr[:])
                    outs = cspool.tile([2, 384], F32, tag="outs", name="outs")
                    nc.vector.tensor_copy(outs[:], rp_)
                    nc.sync.dma_start(out_d[g], outs[:])

                gen = chain_gen(0)
                for _ in gen:
                    pass
                gens = {}
                for it in range(NS + 3):
                    if it < NS:
                        hg_next = it // 8 + 1
                        if hg_next < NHG:
                            if it % 8 == 0:
                                gens[hg_next] = chain_gen(hg_next)
                            next(gens[hg_next], None)
                        stage_T(it)
                    if 1 <= it <= NS:
                        stage_L0(it - 1)
                    if 2 <= it <= NS + 1:
                        stage_L1(it - 2)
                    if 3 <= it <= NS + 2:
                        stage_L2(it - 3)

    _split_waits(nc, mybir)
    return nc


def _host_prep(origins, directions, t_rand, W0, b0, W1, b1, W2, b2):
    """Build per-core input maps (all numpy, cheap)."""
    f32 = np.float32
    # F-row order: rows 3l+c = sin freq l coord c; 18+3l+c = cos; 36..38 pts
    perm = np.zeros(39, np.int64)
    perm[36:39] = (0, 1, 2)
    for l in range(L):
        for c in range(3):
            perm[3 * l + c] = 3 + 6 * l + c
            perm[18 + 3 * l + c] = 3 + 6 * l + 3 + c
    w0p = np.ascontiguousarray(W0[perm]).astype(np.float16)
    w0rep = np.zeros((128, 256), np.float16)
    w0rep[0:39] = w0p
    w0rep[64:103] = w0p

    w2h = np.empty((128, 8), np.float16)
    w2h[:, 0:4] = W2[0:128].astype(np.float16)
    w2h[:, 4:8] = W2[128:256].astype(np.float16)
    b0t = np.ascontiguousarray(b0.reshape(2, 128).T).astype(f32)
    b1t = np.ascontiguousarray(b1.reshape(2, 128).T).astype(f32)
    b2t = np.broadcast_to(b2.astype(f32), (128, 4)).copy()

    q = np.arange(128)
    rp = q // 64
    s = q % 64
    zcpp = (NEAR + DELTA * s).astype(f32).reshape(128, 1).copy()

    # ltri: cols 0..127 exclusive, 128..255 inclusive
    # ltri[k=(rp',j), m=(rp,s)] = (rp'==rp) & (j < s)  /  (j <= s)
    kk = q
    krp = kk // 64
    kj = kk % 64
    same = (krp[:, None] == rp[None, :])
    ltri = np.zeros((128, 256), f32)
    ltri[:, 0:128] = (same & (kj[:, None] < s[None, :])).astype(f32)
    ltri[:, 128:256] = (same & (kj[:, None] <= s[None, :])).astype(f32)
    sel2 = (krp[:, None] == np.arange(2)[None, :]).astype(f32)
    ident = np.eye(128, dtype=f32)
    identh = np.eye(128, dtype=np.float16)

    # ray_of[J, rp] = 16*(J%128) + 2*(J//128) + rp
    J = np.arange(NBLK)
    ray_of = (16 * (J % 128))[:, None] + (2 * (J // 128))[:, None] + np.arange(2)[None, :]

    in_maps = []
    for core in range(NCORES):
        o = origins[core * BC : (core + 1) * BC].astype(f32)
        d = directions[core * BC : (core + 1) * BC].astype(f32)
        t = t_rand[core * BC : (core + 1) * BC].astype(f32)
        tnat = np.ascontiguousarray(t.reshape(128, 1024))
        # aexp[c, q, J] = o[ray_of[J, rp(q)], c]
        rays_qJ = ray_of[:, :].T[rp]  # [128, NBLK] -> rays_qJ[q, J] = ray_of[J, rp[q]]
        aexp = np.ascontiguousarray(o[rays_qJ].transpose(2, 0, 1))
        bexp = np.ascontiguousarray(d[rays_qJ].transpose(2, 0, 1))
        in_maps.append(
            {
                "tnat": tnat,
                "aexp": aexp,
                "bexp": bexp,
                "w0rep": w0rep,
                "w1": W1.astype(np.float16),
                "w2h": w2h,
                "b0t": b0t,
                "b1t": b1t,
                "b2t": b2t,
                "zcpp": zcpp,
                "ltri": ltri,
                "sel2": sel2,
                "ident": ident,
                "identh": identh,
            }
        )
    return in_maps, ray_of


def kernel(origins, directions, t_rand, W0, b0, W1, b1, W2, b2, near, far,
           **kw):
    assert int(near) == 2 and int(far) == 6
    from concourse.bass_utils import run_bass_kernel_spmd

    if "nc" not in _CACHE:
        _CACHE["nc"] = _build()
    nc = _CACHE["nc"]

    in_maps, ray_of = _host_prep(
        np.asarray(origins), np.asarray(directions), np.asarray(t_rand),
        np.asarray(W0), np.asarray(b0), np.asarray(W1), np.asarray(b1),
        np.asarray(W2), np.asarray(b2),
    )
    res = run_bass_kernel_spmd(
        nc, in_maps, core_ids=list(range(NCORES)), trace=PROFILE
    )
    _CACHE["last_results"] = res
    out = np.empty((B, 3), np.float32)
    for core in range(NCORES):
        oc = res.results[core]["out"].reshape(NGRP, 2, 128, 3)
        # group g holds blocks J = 128*g + i ; ray = 16*i + 2*g + rp
        for g in range(NGRP):
            for rpp in range(2):
                rays = core * BC + 16 * np.arange(128) + 2 * g + rpp
                out[rays] = oc[g, rpp]
    return out
